# revision 50
# baseline (speedup 1.0000x reference)
"""Causal self-attention (B=4, T=2048, C=1024, H=16) on 8 trn2 NeuronCores.

Sharding: core = (batch b, head-group hg) -> 4 x 2 grid. Each core computes
attention for 8 of the 16 heads of one batch plus the partial output
projection over its heads' columns; the host sums the two partials per batch
and adds b_proj.

Device-side layout (PE engine cost = streamed-N only; K/M are free):
  - host supplies x^T [C, T] and W^T slices in bf16
  - q,k produced transposed [d, t]; v natural [t, d] with a ones column
    (M = 65) that accumulates the softmax denominator
  - scores S^T [tk, tq]: lhsT = k (K=d=64), rhs = q, N = queries. This is
    N-optimal (score elements / 128).
  - exp on ACT (scale fused); causal diagonal chunks get narrowed matmuls
    plus one [128,128] triangular mask multiply
  - E@v uses the e-slices as the STATIONARY operand: out y [128 queries, 65]
    accumulates over key chunks with N=65 per step (vs N=512 in the naive
    orientation) -- halves attention-phase PE time
  - normalize: DVE reciprocal of the denominator column + broadcast multiply
    straight out of PSUM into y_nat bf16
  - y_nat [q, c] -> yt [c, q] via DMA-transpose (XBAR, 112ns/128x128 tile);
    the tail-critical blocks of the last pair use a PE transpose instead to
    dodge the ~1.7us DMA semaphore latency
  - output projection accumulates yt @ wp per 128-token tile; partials are
    shipped as bf16 (the host sums the two partials per batch in f32)
  - schedule: projection / output-projection groups are spliced between
    attention chunks per-slab to cover the exp (ACT) serial backlog; the
    last slab gets qkproj(3)+vproj(12..15)+outproj(2); the last pair of the
    last slab normalizes per-128-query-slice so outproj(3) can start before
    the final chunk retires.
"""

import sys

if "/opt/trn_rl_repo" not in sys.path:
    sys.path.insert(0, "/opt/trn_rl_repo")

from contextlib import ExitStack

import ml_dtypes
import numpy as np

import concourse.bass as bass
import concourse.mybir as mybir
import concourse.tile as tile
from concourse import bacc
from concourse._compat import with_exitstack
from concourse.bass_utils import run_bass_kernel_spmd

BF16 = mybir.dt.bfloat16
F32 = mybir.dt.float32
NPBF16 = ml_dtypes.bfloat16

B, T, C, H = 4, 2048, 1024, 16
D = C // H              # 64
HPC = 8                 # heads per core
NPAIR = HPC // 2        # head pairs per core
NCORES = 8
CC = C // 128           # 8 contraction chunks
NQT = T // 512          # 4 query slabs of 512
NTT = T // 128          # 16 token tiles of 128
SCALE = 1.0 / np.sqrt(D)


@with_exitstack
def _attention_body(ctx: ExitStack, tc: tile.TileContext, t_in: dict, t_out):
    nc = tc.nc
    consts = ctx.enter_context(tc.tile_pool(name="consts", bufs=1))
    qkp = ctx.enter_context(tc.tile_pool(name="qkp", bufs=1))
    vp = ctx.enter_context(tc.tile_pool(name="vp", bufs=1))
    ytp = ctx.enter_context(tc.tile_pool(name="ytp", bufs=1))
    ynp = ctx.enter_context(tc.tile_pool(name="ynp", bufs=2))
    ep = ctx.enter_context(tc.tile_pool(name="ep", bufs=6))
    rp = ctx.enter_context(tc.tile_pool(name="rp", bufs=4))
    outp = ctx.enter_context(tc.tile_pool(name="outp", bufs=4))
    mm_ps = ctx.enter_context(tc.tile_pool(name="mm_ps", bufs=2, space="PSUM"))
    s_ps = ctx.enter_context(tc.tile_pool(name="s_ps", bufs=2, space="PSUM"))
    y_ps = ctx.enter_context(tc.tile_pool(name="y_ps", bufs=1, space="PSUM"))

    # ---- constants / inputs to SBUF ----
    # Fused multi-chunk tiles: one DMA instruction covers many [128, .] chunks
    # (HWDGE descriptor-gen costs 625ns per DMA instruction, so instruction
    # count -- not bytes -- dominates the startup critical path).
    xTt = consts.tile([128, CC, T], BF16, tag="xT")
    wvt = consts.tile([128, CC, 512], BF16, tag="wv")
    wqkt = consts.tile([128, CC, 1024], BF16, tag="wqk")
    wpt = consts.tile([128, NPAIR, 1024], BF16, tag="wp")
    xT = [xTt[:, c, :] for c in range(CC)]
    wv = [wvt[:, c, :] for c in range(CC)]
    wqk = [wqkt[:, c, :] for c in range(CC)]
    wp = [wpt[:, j, :] for j in range(NPAIR)]
    bqk = consts.tile([128, 8], F32, tag="bqk")
    bv_row = consts.tile([1, 512], F32, tag="bv_row")
    bv_bc = consts.tile([128, 512], F32, tag="bv_bc")
    trieye = consts.tile([128, 2, 128], BF16, tag="trieye")
    tri = trieye[:, 0, :]
    eye = trieye[:, 1, :]

    # DMA order follows consumption: wv + x slab 0 (unlocks vproj 0-3) in
    # growing pieces, tiny consts, wqk (qkproj slab 0), remaining x, wp.
    xTd = t_in["xT"].rearrange("(c p) t -> p c t", p=128)
    wvd = t_in["wvT"].rearrange("(c p) n -> p c n", p=128)
    wqkd = t_in["wqkT"].rearrange("(c p) n -> p c n", p=128)
    wpd = t_in["wpT"].rearrange("(j p) n -> p j n", p=128)
    for lo, hi in ((0, 1), (1, 3), (3, 5), (5, 8)):
        nc.sync.dma_start(wvt[:, lo:hi, :], wvd[:, lo:hi, :])
        nc.sync.dma_start(xTt[:, lo:hi, 0:512], xTd[:, lo:hi, 0:512])
    nc.sync.dma_start(bv_row[:], t_in["bv"][:])
    nc.sync.dma_start(bqk[:], t_in["bqk"][:])
    nc.sync.dma_start(trieye[:], t_in["trieye"][:].rearrange("p (i n) -> p i n", i=2))
    nc.sync.dma_start(wqkt[:, 0:4, :], wqkd[:, 0:4, :])
    nc.sync.dma_start(wqkt[:, 4:8, :], wqkd[:, 4:8, :])
    nc.sync.dma_start(xTt[:, :, 512:2048], xTd[:, :, 512:2048])
    nc.sync.dma_start(wpt[:], wpd[:])
    nc.gpsimd.partition_broadcast(bv_bc[:], bv_row[:])

    qk = [qkp.tile([128, T], BF16, tag=f"qk{j}", name=f"qk{j}") for j in range(CC)]
    v = [vp.tile([128, HPC, D + 1], BF16, tag=f"v{i}", name=f"v{i}") for i in range(NTT)]
    for i in range(NTT):
        nc.vector.memset(v[i][:, :, D:D + 1], 1.0)
    yt = [ytp.tile([128, T], BF16, tag=f"yt{j}", name=f"yt{j}") for j in range(NPAIR)]
    y_nat = {}

    def vproj_group(i):
        ps = mm_ps.tile([128, 512], F32, tag="mm", name="ps_v")
        for cc in range(CC):
            nc.tensor.matmul(
                ps[:],
                xT[cc][:, i * 128:(i + 1) * 128],
                wv[cc][:],
                start=(cc == 0),
                stop=(cc == CC - 1),
            )
        nc.vector.tensor_add(
            v[i][:, :, 0:D],
            ps[:].rearrange("p (h d) -> p h d", h=HPC),
            bv_bc[:].rearrange("p (h d) -> p h d", h=HPC),
        )

    def qkproj_group(oc, tt):
        ps = mm_ps.tile([128, 512], F32, tag="mm", name="ps_qk")
        for cc in range(CC):
            nc.tensor.matmul(
                ps[:],
                wqk[cc][:, oc * 128:(oc + 1) * 128],
                xT[cc][:, tt * 512:(tt + 1) * 512],
                start=(cc == 0),
                stop=(cc == CC - 1),
            )
        nc.vector.tensor_scalar_add(
            qk[oc][:, tt * 512:(tt + 1) * 512], ps[:], bqk[:, oc:oc + 1]
        )

    def geom(qt, kc):
        m = kc - 4 * qt  # >= 0 on diagonal chunks
        qoff = 128 * m if m > 0 else 0
        return m, qoff, 512 - qoff

    def scores_chunk(qt, hp, kc):
        """Scores + exp + mask for one (head-pair, key-chunk); returns e."""
        q0 = qt * 512
        m, qoff, nw = geom(qt, kc)
        k0 = kc * 128
        meng = nc.vector
        sps = s_ps.tile([128, 1024], F32, tag="sps", name="sps")
        nc.tensor.matmul(
            sps[:, 0:nw],
            qk[4 + hp][0:64, k0:k0 + 128],
            qk[hp][0:64, q0 + qoff:q0 + 512],
            start=True, stop=True,
        )
        nc.tensor.matmul(
            sps[:, 512:512 + nw],
            qk[4 + hp][64:128, k0:k0 + 128],
            qk[hp][64:128, q0 + qoff:q0 + 512],
            start=True, stop=True,
        )
        e = ep.tile([128, 1024], BF16, tag="e", name="e")
        nc.scalar.activation(
            e[:].rearrange("p (i n) -> p i n", i=2)[:, :, 0:nw],
            sps[:].rearrange("p (i n) -> p i n", i=2)[:, :, 0:nw],
            mybir.ActivationFunctionType.Exp,
            scale=float(SCALE),
        )
        if m >= 0:
            meng.tensor_mul(
                e[:].rearrange("p (i n) -> p i n", i=2)[:, :, 0:128],
                e[:].rearrange("p (i n) -> p i n", i=2)[:, :, 0:128],
                tri[:].unsqueeze(1).broadcast_to([128, 2, 128]),
            )
        return e

    def ev_chunk(qt, hp, kc, ya, yb, e):
        """E@v with e as the stationary operand: per (head, 128-query slice)
        accumulate [128 q, 65] over key chunks; N=65 per matmul.

        PSUM start=True lazily zeroes the whole 2KB bank, so exactly ONE
        start per tile (first write poisons the bank; the other regions'
        first writes land on pending-zero bytes) and ONE stop on the bank's
        final write."""
        m, qoff, _ = geom(qt, kc)
        for h2, ydst in ((0, ya), (1, yb)):
            for qs in range(max(m, 0), 4):
                c0 = h2 * 512 + qs * 128 - qoff
                nc.tensor.matmul(
                    ydst[:, qs, :],
                    e[:, c0:c0 + 128],
                    v[kc][:, 2 * hp + h2, :],
                    start=(kc == 0 and qs == 0),
                    stop=(kc == 4 * qt + 3 and qs == 3),
                )

    def norm_pair(qt, hp, ya, yb):
        """Batched normalize of a whole pair: reciprocal of the denominator
        columns + broadcast multiply, PSUM -> y_nat bf16."""
        for h2, ysrc in ((0, ya), (1, yb)):
            h = 2 * hp + h2
            r = rp.tile([128, 4], F32, tag="r", name="r")
            nc.vector.reciprocal(r[:], ysrc[:, :, D])
            nc.vector.tensor_mul(
                y_nat[qt][:, :, h * D:(h + 1) * D],
                ysrc[:, :, 0:D],
                r[:].unsqueeze(2).broadcast_to([128, 4, D]),
            )

    def norm_qs(qt, hp, qs, ya, yb, on_act=False):
        """Per-query-slice normalize (tail of the last pair). The very last
        slice multiplies on ACT (idle once the exp chain drains) via an
        activation Copy with a per-partition scale."""
        for h2, ysrc in ((0, ya), (1, yb)):
            h = 2 * hp + h2
            r = rp.tile([128, 1], F32, tag="r1", name="r1")
            nc.vector.reciprocal(r[:], ysrc[:, qs, D:D + 1])
            if on_act:
                nc.scalar.activation(
                    y_nat[qt][:, qs, h * D:(h + 1) * D],
                    ysrc[:, qs, 0:D],
                    mybir.ActivationFunctionType.Copy,
                    scale=r[:],
                )
            else:
                nc.vector.tensor_mul(
                    y_nat[qt][:, qs, h * D:(h + 1) * D],
                    ysrc[:, qs, 0:D],
                    r[:].broadcast_to([128, D]),
                )

    def tdma(qt, qs, j):
        """yt[j] gets the transposed 128x128 block via the DMA XBAR."""
        nc.sync.dma_start(
            yt[j][:, qt * 512 + qs * 128:qt * 512 + (qs + 1) * 128],
            y_nat[qt][:, qs, j * 128:(j + 1) * 128],
            transpose=True,
        )

    def t_pe(qt, qs, j, on_act=False):
        """PE transpose + copy: lower latency than the DMA XBAR path."""
        ps = mm_ps.tile([128, 1024], BF16, tag="mm", name="ps_t")
        nc.tensor.transpose(ps[:, 0:128], y_nat[qt][:, qs, j * 128:(j + 1) * 128],
                            eye[:])
        dst = yt[j][:, qt * 512 + qs * 128:qt * 512 + (qs + 1) * 128]
        if on_act:
            nc.scalar.activation(dst, ps[:, 0:128],
                                 mybir.ActivationFunctionType.Copy)
        else:
            nc.vector.tensor_copy(dst, ps[:, 0:128])

    obuf = {}
    partials = {}

    def outproj_prerun(i, oh):
        """Accumulate pairs 0..2 of the final slab's output projection while
        the last pair's exp chain still runs; only the j=3 matmul and a DVE
        add remain after the last exp."""
        ps = mm_ps.tile([128, 512], F32, tag="mm", name="ps_pr")
        for j in range(NPAIR - 1):
            nc.tensor.matmul(
                ps[:],
                yt[j][:, i * 128:(i + 1) * 128],
                wp[j][:, oh * 512:(oh + 1) * 512],
                start=(j == 0),
                stop=(j == NPAIR - 2),
            )
        if i not in partials:
            partials[i] = outp.tile([128, 1024], BF16, tag="pp", name=f"pp{i}",
                                    bufs=4)
        nc.vector.tensor_copy(partials[i][:, oh * 512:(oh + 1) * 512], ps[:])

    def outproj_tail(i):
        """j3 matmul + partial add + ship, for one 128-token tile. The final
        tile (emitted after the last scores chunk) borrows a free scores-psum
        slot so both halves land in one [128,1024] tile -- one add, one DMA;
        earlier tiles would steal the slot from the still-running exp
        pipeline, so they go per-half through the mm rotation."""
        if i not in obuf:
            obuf[i] = outp.tile([128, 1024], BF16, tag="ob", name=f"ob{i}")
        ob = obuf[i]
        for oh in range(2):
            ps = mm_ps.tile([128, 512], F32, tag="mm", name="ps_tl")
            nc.tensor.matmul(
                ps[:],
                yt[NPAIR - 1][:, i * 128:(i + 1) * 128],
                wp[NPAIR - 1][:, oh * 512:(oh + 1) * 512],
                start=True, stop=True,
            )
            nc.vector.tensor_add(ob[:, oh * 512:(oh + 1) * 512], ps[:],
                                 partials[i][:, oh * 512:(oh + 1) * 512])
            nc.sync.dma_start(
                t_out[i * 128:(i + 1) * 128, oh * 512:(oh + 1) * 512],
                ob[:, oh * 512:(oh + 1) * 512],
            )

    def outproj_group(i, oh):
        ps = mm_ps.tile([128, 512], F32, tag="mm", name="ps_op")
        for j in range(NPAIR):
            nc.tensor.matmul(
                ps[:],
                yt[j][:, i * 128:(i + 1) * 128],
                wp[j][:, oh * 512:(oh + 1) * 512],
                start=(j == 0),
                stop=(j == NPAIR - 1),
            )
        if i not in obuf:
            obuf[i] = outp.tile([128, 1024], BF16, tag="ob", name=f"ob{i}")
        ob = obuf[i]
        nc.vector.tensor_copy(ob[:, oh * 512:(oh + 1) * 512], ps[:])
        if i >= 4 * (NQT - 1):
            # final slab: ship each half as soon as its copy lands
            nc.sync.dma_start(
                t_out[i * 128:(i + 1) * 128, oh * 512:(oh + 1) * 512],
                ob[:, oh * 512:(oh + 1) * 512],
            )
        elif oh == 1:
            nc.sync.dma_start(t_out[i * 128:(i + 1) * 128, :], ob[:])

    # ---- schedule ----
    # Fill units per slab, sized to the slab's exp-vs-PE deficit. Each item
    # is (due, fn): the unit MUST be emitted before chunk-step `due` of its
    # slab (PE executes in program order, so a consumer emitted before its
    # producer deadlocks); due=None means "any time, flush by slab end".
    # Dues: vproj(g) feeds EV at pair-0 step kc=g; qkproj(oc=hp, tt) feeds
    # the one-ahead scores lookahead at step hp*L-1 of slab tt; k-halves
    # (oc=4+hp) feed the lookahead of pair hp's first slab-tt key chunk.
    def L(qt):
        return 4 * qt + 4

    fills = {
        0: [(15 if oc == 0 else None, lambda oc=oc: qkproj_group(oc, 1))
            for oc in range(CC)],
        1: ([(g, lambda i=i: vproj_group(i)) for g, i in
             zip(range(4, 8), range(4, 8))]
            + [(31 if oc == 0 else None, lambda oc=oc: qkproj_group(oc, 2))
               for oc in range(CC)]),
        2: ([(g, lambda i=i: vproj_group(i)) for g, i in
             zip(range(8, 12), range(8, 12))]
            + [(None, lambda i=i, oh=oh: outproj_group(i, oh))
               for i in range(0, 8) for oh in range(2)]
            + [(47, lambda: qkproj_group(0, 3))]),
        3: ([(11, lambda: qkproj_group(4, 3))]
            + [(g, lambda i=i: vproj_group(i)) for g, i in
               zip(range(12, 16), range(12, 16))]
            + [(hp * 16 - 1, lambda oc=hp: qkproj_group(oc, 3))
               for hp in (1, 2, 3)]
            + [(hp * 16 + 11, lambda oc=hp: qkproj_group(4 + oc, 3))
               for hp in (1, 2, 3)]),
    }
    # Late fill for slab 3's back half, where the fill deficit (ACT-paced
    # chunks vs small diagonal matmuls) is otherwise uncovered: outproj(2)
    # (ready since slab-2 end) through pairs 2-3, and the outproj(3)
    # pre-runs once pair 2's transposes have landed (after step 47).
    late_fills = {qt: [] for qt in range(NQT)}
    late_fills[3] = sorted(
        [(34 + 2 * k, fn) for k, fn in enumerate(
            lambda i=i, oh=oh: outproj_group(i, oh)
            for i in range(8, 12) for oh in range(2))]
        + [(50 + (3 * k) // 2, fn) for k, fn in enumerate(
            lambda i=i, oh=oh: outproj_prerun(i, oh)
            for i in range(12, 16) for oh in range(2))],
        key=lambda p: p[0],
    )

    # Global chunk sequence; scores are pipelined one chunk ahead across pair
    # AND slab boundaries so the PE always has the next scores queued while
    # ACT digests the exp backlog.
    chunks = [(qt, hp, kc)
              for qt in range(NQT)
              for hp in range(NPAIR)
              for kc in range(4 * qt + 4)]
    nxt = {chunks[n]: chunks[n + 1] for n in range(len(chunks) - 1)}

    state = {}

    def chunk_step(qt, hp, kc):
        if not state:
            state["e"] = scores_chunk(qt, hp, kc)
        e = state["e"]
        if (qt, hp, kc) in nxt:
            state["e"] = scores_chunk(*nxt[(qt, hp, kc)])
        if kc == 0:
            state["y"] = (
                y_ps.tile([128, 4, D + 1], F32, tag="ya", name="ya", bufs=1),
                y_ps.tile([128, 4, D + 1], F32, tag="yb", name="yb", bufs=1),
            )
        ev_chunk(qt, hp, kc, state["y"][0], state["y"][1], e)

    def norm_qs(qt, hp, qs, ya, yb, on_act=False):
        """Per-query-slice normalize (tail of the last pair). The final
        slice multiplies on ACT (idle once the exp chain drains) via an
        activation Copy with a per-partition scale."""
        for h2, ysrc in ((0, ya), (1, yb)):
            h = 2 * hp + h2
            r = rp.tile([128, 1], F32, tag="r1", name="r1", bufs=8)
            nc.vector.reciprocal(r[:], ysrc[:, qs, D:D + 1])
            if on_act:
                nc.scalar.activation(
                    y_nat[qt][:, qs, h * D:(h + 1) * D],
                    ysrc[:, qs, 0:D],
                    mybir.ActivationFunctionType.Copy,
                    scale=r[:],
                )
            else:
                nc.vector.tensor_mul(
                    y_nat[qt][:, qs, h * D:(h + 1) * D],
                    ysrc[:, qs, 0:D],
                    r[:].broadcast_to([128, D]),
                )

    def post_chunk(qt, hp, kc):
        """Normalize/transpose work to emit right after a chunk. The last
        pair of the last slab staggers its per-slice tail: normalize at its
        diagonal chunk, transpose one chunk later, j3+add two chunks later,
        so each cross-engine chain's latency hides behind the next chunks
        instead of head-of-line blocking the in-order queues."""
        last_pair = (qt == NQT - 1 and hp == NPAIR - 1)
        if last_pair:
            qs = kc - 4 * qt
            if qs < 0:
                return
            norm_qs(qt, hp, qs, *state["y"], on_act=(qs == 3))
            t_pe(qt, qs, hp, on_act=(qs == 3))
            outproj_tail(4 * qt + qs)
        elif kc == 4 * qt + 3:
            if qt not in y_nat:
                y_nat[qt] = ynp.tile([128, 4, 512], BF16, tag="yn",
                                     name=f"yn{qt}")
            norm_pair(qt, hp, *state["y"])
            for qs in range(4):
                tdma(qt, qs, hp)

    # prologue: vproj for slab-0 tokens, then qkproj slab 0
    for i in range(4):
        vproj_group(i)
    for oc in range(CC):
        qkproj_group(oc, 0)

    # main loop: per slab, interleave attention chunks with the fill list.
    # Forced emission for due items; otherwise paced evenly across the slab.
    for qt in range(NQT):
        steps = [(hp, kc) for hp in range(NPAIR) for kc in range(4 * qt + 4)]
        nstep = len(steps)
        fill = list(fills[qt])
        late = list(late_fills[qt])
        span_e = max(1, int(nstep * (0.62 if late else 1.0)))
        total = len(fill)
        emitted = 0
        for n, (hp, kc) in enumerate(steps):
            # forced: everything that must precede this chunk step
            keep = []
            for due, fn in fill:
                if due is not None and due <= n:
                    fn()
                    emitted += 1
                else:
                    keep.append((due, fn))
            fill = keep
            chunk_step(qt, hp, kc)
            post_chunk(qt, hp, kc)
            # paced: early list across the first span_e steps
            target = min(total, (total * (n + 1) + span_e - 1) // span_e)
            while emitted < target and fill:
                due, fn = fill.pop(0)
                fn()
                emitted += 1
            # late list: each item fires at its start step
            while late and late[0][0] <= n:
                late.pop(0)[1]()
        for due, fn in fill + late:
            fn()

    if t_in.get("dbg"):
        for j in range(NPAIR):
            nc.sync.dma_start(t_in["dbg_yt"][j * 128:(j + 1) * 128, :], yt[j][:])
        for qt in range(NQT):
            nc.sync.dma_start(
                t_in["dbg_yn"][qt * 128:(qt + 1) * 128, :],
                y_nat[qt][:].rearrange("p a b -> p (a b)"),
            )


def build_model():
    nc = bacc.Bacc(
        "TRN2",
        target_bir_lowering=False,
        debug=False,
        enable_asserts=False,
        num_devices=NCORES,
    )
    t_in = {
        "xT": nc.dram_tensor("xT", [C, T], BF16, kind="ExternalInput").ap(),
        "wqkT": nc.dram_tensor("wqkT", [C, 1024], BF16, kind="ExternalInput").ap(),
        "wvT": nc.dram_tensor("wvT", [C, 512], BF16, kind="ExternalInput").ap(),
        "wpT": nc.dram_tensor("wpT", [512, C], BF16, kind="ExternalInput").ap(),
        "bqk": nc.dram_tensor("bqk", [128, 8], F32, kind="ExternalInput").ap(),
        "bv": nc.dram_tensor("bv", [1, 512], F32, kind="ExternalInput").ap(),
        "trieye": nc.dram_tensor("trieye", [128, 256], BF16, kind="ExternalInput").ap(),
    }
    t_out = nc.dram_tensor("out", [T, C], BF16, kind="ExternalOutput").ap()
    with tile.TileContext(nc) as tc:
        _attention_body(tc, t_in, t_out)
    nc.compile()
    return nc


def make_in_maps(x, w_attn, b_attn, w_proj):
    """Host-side sharding: per-core input dict for core (b, hg)."""
    trieye = np.concatenate(
        [np.triu(np.ones((128, 128), np.float32)), np.eye(128, dtype=np.float32)],
        axis=1,
    ).astype(NPBF16)
    in_maps = []
    xT_cache = {}
    for cid in range(NCORES):
        b, hg = cid // 2, cid % 2
        h0 = hg * HPC
        if b not in xT_cache:
            xT_cache[b] = np.ascontiguousarray(x[b].T).astype(NPBF16)
        rq = slice(h0 * D, (h0 + HPC) * D)
        rk = slice(C + h0 * D, C + (h0 + HPC) * D)
        rv = slice(2 * C + h0 * D, 2 * C + (h0 + HPC) * D)
        wqkT = np.ascontiguousarray(
            np.concatenate([w_attn[rq], w_attn[rk]], axis=0).T
        ).astype(NPBF16)
        wvT = np.ascontiguousarray(w_attn[rv].T).astype(NPBF16)
        wpT = np.ascontiguousarray(w_proj[:, h0 * D:(h0 + HPC) * D].T).astype(NPBF16)
        bqk = np.stack(
            [b_attn[rq].reshape(4, 128)[j] for j in range(4)]
            + [b_attn[rk].reshape(4, 128)[j] for j in range(4)],
            axis=1,
        ).astype(np.float32)
        bv = b_attn[rv].reshape(1, 512).astype(np.float32)
        in_maps.append({
            "xT": xT_cache[b],
            "wqkT": wqkT,
            "wvT": wvT,
            "wpT": wpT,
            "bqk": np.ascontiguousarray(bqk),
            "bv": bv,
            "trieye": trieye,
        })
    return in_maps


_NC_CACHE = []


def kernel(x, w_attn, b_attn, w_proj, b_proj):
    x = np.asarray(x, dtype=np.float32)
    w_attn = np.asarray(w_attn, dtype=np.float32)
    b_attn = np.asarray(b_attn, dtype=np.float32)
    w_proj = np.asarray(w_proj, dtype=np.float32)
    b_proj = np.asarray(b_proj, dtype=np.float32)

    if not _NC_CACHE:
        _NC_CACHE.append(build_model())
    nc = _NC_CACHE[0]
    in_maps = make_in_maps(x, w_attn, b_attn, w_proj)
    res = None
    for attempt in range(3):
        try:
            res = run_bass_kernel_spmd(nc, in_maps, core_ids=list(range(NCORES)))
            break
        except Exception:
            if attempt == 2:
                raise
            import time
            time.sleep(5)
    out = np.empty((B, T, C), np.float32)
    for b in range(B):
        out[b] = (res.results[2 * b]["out"].astype(np.float32)
                  + res.results[2 * b + 1]["out"].astype(np.float32))
    out += b_proj[None, None, :]
    return out


# revision 61
# speedup vs baseline: 1.0025x; 1.0025x over previous
"""Causal self-attention (B=4, T=2048, C=1024, H=16) on 8 trn2 NeuronCores.

Sharding: core = (batch b, head-group hg) -> 4 x 2 grid. Each core computes
attention for 8 of the 16 heads of one batch plus the partial output
projection over its heads' columns; the host sums the two partials per batch
and adds b_proj.

Device-side layout (PE engine cost = streamed-N only; K/M are free):
  - host supplies x^T [C, T] and W^T slices in bf16
  - q,k produced transposed [d, t]; v natural [t, d] with a ones column
    (M = 65) that accumulates the softmax denominator
  - scores S^T [tk, tq]: lhsT = k (K=d=64), rhs = q, N = queries. This is
    N-optimal (score elements / 128).
  - exp on ACT (scale fused); causal diagonal chunks get narrowed matmuls
    plus one [128,128] triangular mask multiply
  - E@v uses the e-slices as the STATIONARY operand: out y [128 queries, 65]
    accumulates over key chunks with N=65 per step (vs N=512 in the naive
    orientation) -- halves attention-phase PE time
  - normalize: DVE reciprocal of the denominator column + broadcast multiply
    straight out of PSUM into y_nat bf16
  - y_nat [q, c] -> yt [c, q] via DMA-transpose (XBAR, 112ns/128x128 tile);
    the tail-critical blocks of the last pair use a PE transpose instead to
    dodge the ~1.7us DMA semaphore latency
  - output projection accumulates yt @ wp per 128-token tile; partials are
    shipped as bf16 (the host sums the two partials per batch in f32)
  - schedule: projection / output-projection groups are spliced between
    attention chunks per-slab to cover the exp (ACT) serial backlog; the
    last slab gets qkproj(3)+vproj(12..15)+outproj(2); the last pair of the
    last slab normalizes per-128-query-slice so outproj(3) can start before
    the final chunk retires.
"""

import sys

if "/opt/trn_rl_repo" not in sys.path:
    sys.path.insert(0, "/opt/trn_rl_repo")

from contextlib import ExitStack

import ml_dtypes
import numpy as np

import concourse.bass as bass
import concourse.mybir as mybir
import concourse.tile as tile
from concourse import bacc
from concourse._compat import with_exitstack
from concourse.bass_utils import run_bass_kernel_spmd

BF16 = mybir.dt.bfloat16
F32 = mybir.dt.float32
NPBF16 = ml_dtypes.bfloat16

B, T, C, H = 4, 2048, 1024, 16
D = C // H              # 64
HPC = 8                 # heads per core
NPAIR = HPC // 2        # head pairs per core
NCORES = 8
CC = C // 128           # 8 contraction chunks
NQT = T // 512          # 4 query slabs of 512
NTT = T // 128          # 16 token tiles of 128
SCALE = 1.0 / np.sqrt(D)


@with_exitstack
def _attention_body(ctx: ExitStack, tc: tile.TileContext, t_in: dict, t_out):
    nc = tc.nc
    consts = ctx.enter_context(tc.tile_pool(name="consts", bufs=1))
    qkp = ctx.enter_context(tc.tile_pool(name="qkp", bufs=1))
    vp = ctx.enter_context(tc.tile_pool(name="vp", bufs=1))
    ytp = ctx.enter_context(tc.tile_pool(name="ytp", bufs=1))
    ynp = ctx.enter_context(tc.tile_pool(name="ynp", bufs=2))
    ep = ctx.enter_context(tc.tile_pool(name="ep", bufs=6))
    rp = ctx.enter_context(tc.tile_pool(name="rp", bufs=4))
    outp = ctx.enter_context(tc.tile_pool(name="outp", bufs=4))
    mm_ps = ctx.enter_context(tc.tile_pool(name="mm_ps", bufs=2, space="PSUM"))
    s_ps = ctx.enter_context(tc.tile_pool(name="s_ps", bufs=2, space="PSUM"))
    y_ps = ctx.enter_context(tc.tile_pool(name="y_ps", bufs=1, space="PSUM"))

    # ---- constants / inputs to SBUF ----
    # Fused multi-chunk tiles: one DMA instruction covers many [128, .] chunks
    # (HWDGE descriptor-gen costs 625ns per DMA instruction, so instruction
    # count -- not bytes -- dominates the startup critical path).
    xTt = consts.tile([128, CC, T], BF16, tag="xT")
    wvt = consts.tile([128, CC, 512], BF16, tag="wv")
    wqkt = consts.tile([128, CC, 1024], BF16, tag="wqk")
    wpt = consts.tile([128, NPAIR, 1024], BF16, tag="wp")
    xT = [xTt[:, c, :] for c in range(CC)]
    wv = [wvt[:, c, :] for c in range(CC)]
    wqk = [wqkt[:, c, :] for c in range(CC)]
    wp = [wpt[:, j, :] for j in range(NPAIR)]
    bqk = consts.tile([128, 8], F32, tag="bqk")
    bv_row = consts.tile([1, 512], F32, tag="bv_row")
    bv_bc = consts.tile([128, 512], F32, tag="bv_bc")
    trieye = consts.tile([128, 2, 128], BF16, tag="trieye")
    tri = trieye[:, 0, :]
    eye = trieye[:, 1, :]

    # DMA order follows consumption: wv + x slab 0 (unlocks vproj 0-3) in
    # growing pieces, tiny consts, wqk (qkproj slab 0), remaining x, wp.
    xTd = t_in["xT"].rearrange("(c p) t -> p c t", p=128)
    wvd = t_in["wvT"].rearrange("(c p) n -> p c n", p=128)
    wqkd = t_in["wqkT"].rearrange("(c p) n -> p c n", p=128)
    wpd = t_in["wpT"].rearrange("(j p) n -> p j n", p=128)
    for lo, hi in ((0, 1), (1, 3), (3, 5), (5, 8)):
        nc.sync.dma_start(wvt[:, lo:hi, :], wvd[:, lo:hi, :])
        nc.sync.dma_start(xTt[:, lo:hi, 0:512], xTd[:, lo:hi, 0:512])
    nc.sync.dma_start(bv_row[:], t_in["bv"][:])
    nc.sync.dma_start(bqk[:], t_in["bqk"][:])
    nc.sync.dma_start(trieye[:], t_in["trieye"][:].rearrange("p (i n) -> p i n", i=2))
    nc.sync.dma_start(wqkt[:, 0:4, :], wqkd[:, 0:4, :])
    nc.sync.dma_start(wqkt[:, 4:8, :], wqkd[:, 4:8, :])
    nc.sync.dma_start(xTt[:, :, 512:2048], xTd[:, :, 512:2048])
    nc.sync.dma_start(wpt[:], wpd[:])
    nc.gpsimd.partition_broadcast(bv_bc[:], bv_row[:])

    qk = [qkp.tile([128, T], BF16, tag=f"qk{j}", name=f"qk{j}") for j in range(CC)]
    v = [vp.tile([128, HPC, D + 1], BF16, tag=f"v{i}", name=f"v{i}") for i in range(NTT)]
    for i in range(NTT):
        nc.vector.memset(v[i][:, :, D:D + 1], 1.0)
    yt = [ytp.tile([128, T], BF16, tag=f"yt{j}", name=f"yt{j}") for j in range(NPAIR)]
    y_nat = {}

    def vproj_group(i):
        ps = mm_ps.tile([128, 512], F32, tag="mm", name="ps_v")
        for cc in range(CC):
            nc.tensor.matmul(
                ps[:],
                xT[cc][:, i * 128:(i + 1) * 128],
                wv[cc][:],
                start=(cc == 0),
                stop=(cc == CC - 1),
            )
        nc.vector.tensor_add(
            v[i][:, :, 0:D],
            ps[:].rearrange("p (h d) -> p h d", h=HPC),
            bv_bc[:].rearrange("p (h d) -> p h d", h=HPC),
        )

    def qkproj_group(oc, tt):
        ps = mm_ps.tile([128, 512], F32, tag="mm", name="ps_qk")
        for cc in range(CC):
            nc.tensor.matmul(
                ps[:],
                wqk[cc][:, oc * 128:(oc + 1) * 128],
                xT[cc][:, tt * 512:(tt + 1) * 512],
                start=(cc == 0),
                stop=(cc == CC - 1),
            )
        nc.vector.tensor_scalar_add(
            qk[oc][:, tt * 512:(tt + 1) * 512], ps[:], bqk[:, oc:oc + 1]
        )

    def geom(qt, kc):
        m = kc - 4 * qt  # >= 0 on diagonal chunks
        qoff = 128 * m if m > 0 else 0
        return m, qoff, 512 - qoff

    def scores_chunk(qt, hp, kc):
        """Scores + exp + mask for one (head-pair, key-chunk); returns e."""
        q0 = qt * 512
        m, qoff, nw = geom(qt, kc)
        k0 = kc * 128
        meng = nc.vector
        sps = s_ps.tile([128, 1024], F32, tag="sps", name="sps")
        nc.tensor.matmul(
            sps[:, 0:nw],
            qk[4 + hp][0:64, k0:k0 + 128],
            qk[hp][0:64, q0 + qoff:q0 + 512],
            start=True, stop=True,
        )
        nc.tensor.matmul(
            sps[:, 512:512 + nw],
            qk[4 + hp][64:128, k0:k0 + 128],
            qk[hp][64:128, q0 + qoff:q0 + 512],
            start=True, stop=True,
        )
        e = ep.tile([128, 1024], BF16, tag="e", name="e")
        nc.scalar.activation(
            e[:].rearrange("p (i n) -> p i n", i=2)[:, :, 0:nw],
            sps[:].rearrange("p (i n) -> p i n", i=2)[:, :, 0:nw],
            mybir.ActivationFunctionType.Exp,
            scale=float(SCALE),
        )
        if m >= 0:
            meng.tensor_mul(
                e[:].rearrange("p (i n) -> p i n", i=2)[:, :, 0:128],
                e[:].rearrange("p (i n) -> p i n", i=2)[:, :, 0:128],
                tri[:].unsqueeze(1).broadcast_to([128, 2, 128]),
            )
        return e

    def ev_chunk(qt, hp, kc, ya, yb, e):
        """E@v with e as the stationary operand: per (head, 128-query slice)
        accumulate [128 q, 65] over key chunks; N=65 per matmul.

        PSUM start=True lazily zeroes the whole 2KB bank, so exactly ONE
        start per tile (first write poisons the bank; the other regions'
        first writes land on pending-zero bytes) and ONE stop on the bank's
        final write."""
        m, qoff, _ = geom(qt, kc)
        for h2, ydst in ((0, ya), (1, yb)):
            for qs in range(max(m, 0), 4):
                c0 = h2 * 512 + qs * 128 - qoff
                nc.tensor.matmul(
                    ydst[:, qs, :],
                    e[:, c0:c0 + 128],
                    v[kc][:, 2 * hp + h2, :],
                    start=(kc == 0 and qs == 0),
                    stop=(kc == 4 * qt + 3 and qs == 3),
                )

    def norm_pair(qt, hp, ya, yb):
        """Batched normalize of a whole pair: reciprocal of the denominator
        columns + broadcast multiply, PSUM -> y_nat bf16."""
        for h2, ysrc in ((0, ya), (1, yb)):
            h = 2 * hp + h2
            r = rp.tile([128, 4], F32, tag="r", name="r")
            nc.vector.reciprocal(r[:], ysrc[:, :, D])
            nc.vector.tensor_mul(
                y_nat[qt][:, :, h * D:(h + 1) * D],
                ysrc[:, :, 0:D],
                r[:].unsqueeze(2).broadcast_to([128, 4, D]),
            )

    def norm_qs(qt, hp, qs, ya, yb, on_act=False):
        """Per-query-slice normalize (tail of the last pair). The very last
        slice multiplies on ACT (idle once the exp chain drains) via an
        activation Copy with a per-partition scale."""
        for h2, ysrc in ((0, ya), (1, yb)):
            h = 2 * hp + h2
            r = rp.tile([128, 1], F32, tag="r1", name="r1")
            nc.vector.reciprocal(r[:], ysrc[:, qs, D:D + 1])
            if on_act:
                nc.scalar.activation(
                    y_nat[qt][:, qs, h * D:(h + 1) * D],
                    ysrc[:, qs, 0:D],
                    mybir.ActivationFunctionType.Copy,
                    scale=r[:],
                )
            else:
                nc.vector.tensor_mul(
                    y_nat[qt][:, qs, h * D:(h + 1) * D],
                    ysrc[:, qs, 0:D],
                    r[:].broadcast_to([128, D]),
                )

    def tdma(qt, qs, j):
        """yt[j] gets the transposed 128x128 block via the DMA XBAR."""
        nc.sync.dma_start(
            yt[j][:, qt * 512 + qs * 128:qt * 512 + (qs + 1) * 128],
            y_nat[qt][:, qs, j * 128:(j + 1) * 128],
            transpose=True,
        )

    def t_pe(qt, qs, j, on_act=False):
        """PE transpose + copy: lower latency than the DMA XBAR path."""
        ps = mm_ps.tile([128, 1024], BF16, tag="mm", name="ps_t")
        nc.tensor.transpose(ps[:, 0:128], y_nat[qt][:, qs, j * 128:(j + 1) * 128],
                            eye[:])
        dst = yt[j][:, qt * 512 + qs * 128:qt * 512 + (qs + 1) * 128]
        if on_act:
            nc.scalar.activation(dst, ps[:, 0:128],
                                 mybir.ActivationFunctionType.Copy)
        else:
            nc.vector.tensor_copy(dst, ps[:, 0:128])

    obuf = {}
    partials = {}

    def outproj_prerun(i, oh):
        """Accumulate pairs 0..2 of the final slab's output projection while
        the last pair's exp chain still runs; only the j=3 matmul and a DVE
        add remain after the last exp."""
        ps = mm_ps.tile([128, 512], F32, tag="mm", name="ps_pr")
        for j in range(NPAIR - 1):
            nc.tensor.matmul(
                ps[:],
                yt[j][:, i * 128:(i + 1) * 128],
                wp[j][:, oh * 512:(oh + 1) * 512],
                start=(j == 0),
                stop=(j == NPAIR - 2),
            )
        if i not in partials:
            partials[i] = outp.tile([128, 1024], BF16, tag="pp", name=f"pp{i}",
                                    bufs=4)
        nc.vector.tensor_copy(partials[i][:, oh * 512:(oh + 1) * 512], ps[:])

    def outproj_tail(i):
        """j3 matmul + partial add + ship, for one 128-token tile. The final
        tile (emitted after the last scores chunk) borrows a free scores-psum
        slot so both halves land in one [128,1024] tile -- one add, one DMA;
        earlier tiles would steal the slot from the still-running exp
        pipeline, so they go per-half through the mm rotation."""
        if i not in obuf:
            obuf[i] = outp.tile([128, 1024], BF16, tag="ob", name=f"ob{i}")
        ob = obuf[i]
        for oh in range(2):
            ps = mm_ps.tile([128, 512], F32, tag="mm", name="ps_tl")
            nc.tensor.matmul(
                ps[:],
                yt[NPAIR - 1][:, i * 128:(i + 1) * 128],
                wp[NPAIR - 1][:, oh * 512:(oh + 1) * 512],
                start=True, stop=True,
            )
            dst = ob[:, oh * 512:(oh + 1) * 512]
            nc.vector.tensor_add(dst, ps[:],
                                 partials[i][:, oh * 512:(oh + 1) * 512])
            nc.sync.dma_start(
                t_out[i * 128:(i + 1) * 128, oh * 512:(oh + 1) * 512],
                dst,
            )

    def outproj_group(i, oh):
        ps = mm_ps.tile([128, 512], F32, tag="mm", name="ps_op")
        for j in range(NPAIR):
            nc.tensor.matmul(
                ps[:],
                yt[j][:, i * 128:(i + 1) * 128],
                wp[j][:, oh * 512:(oh + 1) * 512],
                start=(j == 0),
                stop=(j == NPAIR - 1),
            )
        if i not in obuf:
            obuf[i] = outp.tile([128, 1024], BF16, tag="ob", name=f"ob{i}")
        ob = obuf[i]
        nc.vector.tensor_copy(ob[:, oh * 512:(oh + 1) * 512], ps[:])
        if i >= 4 * (NQT - 1):
            # final slab: ship each half as soon as its copy lands
            nc.sync.dma_start(
                t_out[i * 128:(i + 1) * 128, oh * 512:(oh + 1) * 512],
                ob[:, oh * 512:(oh + 1) * 512],
            )
        elif oh == 1:
            nc.sync.dma_start(t_out[i * 128:(i + 1) * 128, :], ob[:])

    # ---- schedule ----
    # Fill units per slab, sized to the slab's exp-vs-PE deficit. Each item
    # is (due, fn): the unit MUST be emitted before chunk-step `due` of its
    # slab (PE executes in program order, so a consumer emitted before its
    # producer deadlocks); due=None means "any time, flush by slab end".
    # Dues: vproj(g) feeds EV at pair-0 step kc=g; qkproj(oc=hp, tt) feeds
    # the one-ahead scores lookahead at step hp*L-1 of slab tt; k-halves
    # (oc=4+hp) feed the lookahead of pair hp's first slab-tt key chunk.
    def L(qt):
        return 4 * qt + 4

    fills = {
        0: [(15 if oc == 0 else None, lambda oc=oc: qkproj_group(oc, 1))
            for oc in range(CC)],
        1: ([(g, lambda i=i: vproj_group(i)) for g, i in
             zip(range(4, 8), range(4, 8))]
            + [(31 if oc == 0 else None, lambda oc=oc: qkproj_group(oc, 2))
               for oc in range(CC)]),
        2: ([(g, lambda i=i: vproj_group(i)) for g, i in
             zip(range(8, 12), range(8, 12))]
            + [(None, lambda i=i, oh=oh: outproj_group(i, oh))
               for i in range(0, 8) for oh in range(2)]
            + [(47, lambda: qkproj_group(0, 3))]),
        3: ([(11, lambda: qkproj_group(4, 3))]
            + [(g, lambda i=i: vproj_group(i)) for g, i in
               zip(range(12, 16), range(12, 16))]
            + [(hp * 16 - 1, lambda oc=hp: qkproj_group(oc, 3))
               for hp in (1, 2, 3)]
            + [(hp * 16 + 11, lambda oc=hp: qkproj_group(4 + oc, 3))
               for hp in (1, 2, 3)]),
    }
    # Late fill for slab 3's back half, where the fill deficit (ACT-paced
    # chunks vs small diagonal matmuls) is otherwise uncovered: outproj(2)
    # (ready since slab-2 end) through pairs 2-3, and the outproj(3)
    # pre-runs once pair 2's transposes have landed (after step 47).
    late_fills = {qt: [] for qt in range(NQT)}
    late_fills[3] = sorted(
        [(34 + 2 * k, fn) for k, fn in enumerate(
            lambda i=i, oh=oh: outproj_group(i, oh)
            for i in range(8, 12) for oh in range(2))],
        key=lambda p: p[0],
    )

    # Global chunk sequence; scores are pipelined one chunk ahead across pair
    # AND slab boundaries so the PE always has the next scores queued while
    # ACT digests the exp backlog.
    chunks = [(qt, hp, kc)
              for qt in range(NQT)
              for hp in range(NPAIR)
              for kc in range(4 * qt + 4)]
    nxt = {chunks[n]: chunks[n + 1] for n in range(len(chunks) - 1)}

    state = {}

    def chunk_step(qt, hp, kc):
        if not state:
            state["e"] = scores_chunk(qt, hp, kc)
        e = state["e"]
        if (qt, hp, kc) in nxt:
            state["e"] = scores_chunk(*nxt[(qt, hp, kc)])
        if kc == 0:
            state["y"] = (
                y_ps.tile([128, 4, D + 1], F32, tag="ya", name="ya", bufs=1),
                y_ps.tile([128, 4, D + 1], F32, tag="yb", name="yb", bufs=1),
            )
        ev_chunk(qt, hp, kc, state["y"][0], state["y"][1], e)

    def norm_qs(qt, hp, qs, ya, yb, on_act=False):
        """Per-query-slice normalize (tail of the last pair). The final
        slice multiplies on ACT (idle once the exp chain drains) via an
        activation Copy with a per-partition scale."""
        for h2, ysrc in ((0, ya), (1, yb)):
            h = 2 * hp + h2
            r = rp.tile([128, 1], F32, tag="r1", name="r1", bufs=8)
            nc.vector.reciprocal(r[:], ysrc[:, qs, D:D + 1])
            if on_act:
                nc.scalar.activation(
                    y_nat[qt][:, qs, h * D:(h + 1) * D],
                    ysrc[:, qs, 0:D],
                    mybir.ActivationFunctionType.Copy,
                    scale=r[:],
                )
            else:
                nc.vector.tensor_mul(
                    y_nat[qt][:, qs, h * D:(h + 1) * D],
                    ysrc[:, qs, 0:D],
                    r[:].broadcast_to([128, D]),
                )

    def post_chunk(qt, hp, kc):
        """Normalize/transpose work to emit right after a chunk. The last
        pair of the last slab staggers its per-slice tail: normalize at its
        diagonal chunk, transpose one chunk later, j3+add two chunks later,
        so each cross-engine chain's latency hides behind the next chunks
        instead of head-of-line blocking the in-order queues."""
        last_pair = (qt == NQT - 1 and hp == NPAIR - 1)
        if last_pair:
            qs = kc - 4 * qt
            if qs == -1:
                for i in range(12, 16):
                    for oh in range(2):
                        outproj_prerun(i, oh)
            if qs < 0:
                return
            norm_qs(qt, hp, qs, *state["y"], on_act=(qs == 3))
            t_pe(qt, qs, hp, on_act=(qs == 3))
            outproj_tail(4 * qt + qs)
        elif kc == 4 * qt + 3:
            if qt not in y_nat:
                y_nat[qt] = ynp.tile([128, 4, 512], BF16, tag="yn",
                                     name=f"yn{qt}")
            norm_pair(qt, hp, *state["y"])
            for qs in range(4):
                tdma(qt, qs, hp)

    # prologue: vproj for slab-0 tokens, then qkproj slab 0
    for i in range(4):
        vproj_group(i)
    for oc in range(CC):
        qkproj_group(oc, 0)

    # main loop: per slab, interleave attention chunks with the fill list.
    # Forced emission for due items; otherwise paced evenly across the slab.
    for qt in range(NQT):
        steps = [(hp, kc) for hp in range(NPAIR) for kc in range(4 * qt + 4)]
        nstep = len(steps)
        fill = list(fills[qt])
        late = list(late_fills[qt])
        span_e = max(1, int(nstep * (0.62 if late else 1.0)))
        total = len(fill)
        emitted = 0
        for n, (hp, kc) in enumerate(steps):
            # forced: everything that must precede this chunk step
            keep = []
            for due, fn in fill:
                if due is not None and due <= n:
                    fn()
                    emitted += 1
                else:
                    keep.append((due, fn))
            fill = keep
            chunk_step(qt, hp, kc)
            post_chunk(qt, hp, kc)
            # paced: early list across the first span_e steps
            target = min(total, (total * (n + 1) + span_e - 1) // span_e)
            while emitted < target and fill:
                due, fn = fill.pop(0)
                fn()
                emitted += 1
            # late list: each item fires at its start step
            while late and late[0][0] <= n:
                late.pop(0)[1]()
        for due, fn in fill + late:
            fn()

    if t_in.get("dbg"):
        for j in range(NPAIR):
            nc.sync.dma_start(t_in["dbg_yt"][j * 128:(j + 1) * 128, :], yt[j][:])
        for qt in range(NQT):
            nc.sync.dma_start(
                t_in["dbg_yn"][qt * 128:(qt + 1) * 128, :],
                y_nat[qt][:].rearrange("p a b -> p (a b)"),
            )


def build_model():
    nc = bacc.Bacc(
        "TRN2",
        target_bir_lowering=False,
        debug=False,
        enable_asserts=False,
        num_devices=NCORES,
    )
    t_in = {
        "xT": nc.dram_tensor("xT", [C, T], BF16, kind="ExternalInput").ap(),
        "wqkT": nc.dram_tensor("wqkT", [C, 1024], BF16, kind="ExternalInput").ap(),
        "wvT": nc.dram_tensor("wvT", [C, 512], BF16, kind="ExternalInput").ap(),
        "wpT": nc.dram_tensor("wpT", [512, C], BF16, kind="ExternalInput").ap(),
        "bqk": nc.dram_tensor("bqk", [128, 8], F32, kind="ExternalInput").ap(),
        "bv": nc.dram_tensor("bv", [1, 512], F32, kind="ExternalInput").ap(),
        "trieye": nc.dram_tensor("trieye", [128, 256], BF16, kind="ExternalInput").ap(),
    }
    t_out = nc.dram_tensor("out", [T, C], BF16, kind="ExternalOutput").ap()
    with tile.TileContext(nc) as tc:
        _attention_body(tc, t_in, t_out)
    nc.compile()
    return nc


def make_in_maps(x, w_attn, b_attn, w_proj):
    """Host-side sharding: per-core input dict for core (b, hg)."""
    trieye = np.concatenate(
        [np.triu(np.ones((128, 128), np.float32)), np.eye(128, dtype=np.float32)],
        axis=1,
    ).astype(NPBF16)
    in_maps = []
    xT_cache = {}
    for cid in range(NCORES):
        b, hg = cid // 2, cid % 2
        h0 = hg * HPC
        if b not in xT_cache:
            xT_cache[b] = np.ascontiguousarray(x[b].T).astype(NPBF16)
        rq = slice(h0 * D, (h0 + HPC) * D)
        rk = slice(C + h0 * D, C + (h0 + HPC) * D)
        rv = slice(2 * C + h0 * D, 2 * C + (h0 + HPC) * D)
        wqkT = np.ascontiguousarray(
            np.concatenate([w_attn[rq], w_attn[rk]], axis=0).T
        ).astype(NPBF16)
        wvT = np.ascontiguousarray(w_attn[rv].T).astype(NPBF16)
        wpT = np.ascontiguousarray(w_proj[:, h0 * D:(h0 + HPC) * D].T).astype(NPBF16)
        bqk = np.stack(
            [b_attn[rq].reshape(4, 128)[j] for j in range(4)]
            + [b_attn[rk].reshape(4, 128)[j] for j in range(4)],
            axis=1,
        ).astype(np.float32)
        bv = b_attn[rv].reshape(1, 512).astype(np.float32)
        in_maps.append({
            "xT": xT_cache[b],
            "wqkT": wqkT,
            "wvT": wvT,
            "wpT": wpT,
            "bqk": np.ascontiguousarray(bqk),
            "bv": bv,
            "trieye": trieye,
        })
    return in_maps


_NC_CACHE = []


def kernel(x, w_attn, b_attn, w_proj, b_proj):
    x = np.asarray(x, dtype=np.float32)
    w_attn = np.asarray(w_attn, dtype=np.float32)
    b_attn = np.asarray(b_attn, dtype=np.float32)
    w_proj = np.asarray(w_proj, dtype=np.float32)
    b_proj = np.asarray(b_proj, dtype=np.float32)

    if not _NC_CACHE:
        _NC_CACHE.append(build_model())
    nc = _NC_CACHE[0]
    in_maps = make_in_maps(x, w_attn, b_attn, w_proj)
    res = None
    for attempt in range(3):
        try:
            res = run_bass_kernel_spmd(nc, in_maps, core_ids=list(range(NCORES)))
            break
        except Exception:
            if attempt == 2:
                raise
            import time
            time.sleep(5)
    out = np.empty((B, T, C), np.float32)
    for b in range(B):
        out[b] = (res.results[2 * b]["out"].astype(np.float32)
                  + res.results[2 * b + 1]["out"].astype(np.float32))
    out += b_proj[None, None, :]
    return out


# revision 67
# speedup vs baseline: 1.0042x; 1.0017x over previous
"""Causal self-attention (B=4, T=2048, C=1024, H=16) on 8 trn2 NeuronCores.

Sharding: core = (batch b, head-group hg) -> 4 x 2 grid. Each core computes
attention for 8 of the 16 heads of one batch plus the partial output
projection over its heads' columns; the host sums the two partials per batch
and adds b_proj.

Device-side layout (PE engine cost = streamed-N only; K/M are free):
  - host supplies x^T [C, T] and W^T slices in bf16
  - q,k produced transposed [d, t]; v natural [t, d] with a ones column
    (M = 65) that accumulates the softmax denominator
  - scores S^T [tk, tq]: lhsT = k (K=d=64), rhs = q, N = queries. This is
    N-optimal (score elements / 128).
  - exp on ACT (scale fused); causal diagonal chunks get narrowed matmuls
    plus one [128,128] triangular mask multiply
  - E@v uses the e-slices as the STATIONARY operand: out y [128 queries, 65]
    accumulates over key chunks with N=65 per step (vs N=512 in the naive
    orientation) -- halves attention-phase PE time
  - normalize: DVE reciprocal of the denominator column + broadcast multiply
    straight out of PSUM into y_nat bf16
  - y_nat [q, c] -> yt [c, q] via DMA-transpose (XBAR, 112ns/128x128 tile);
    the tail-critical blocks of the last pair use a PE transpose instead to
    dodge the ~1.7us DMA semaphore latency
  - output projection accumulates yt @ wp per 128-token tile; partials are
    shipped as bf16 (the host sums the two partials per batch in f32)
  - schedule: projection / output-projection groups are spliced between
    attention chunks per-slab to cover the exp (ACT) serial backlog; the
    last slab gets qkproj(3)+vproj(12..15)+outproj(2); the last pair of the
    last slab normalizes per-128-query-slice so outproj(3) can start before
    the final chunk retires.
"""

import sys

if "/opt/trn_rl_repo" not in sys.path:
    sys.path.insert(0, "/opt/trn_rl_repo")

from contextlib import ExitStack

import ml_dtypes
import numpy as np

import concourse.bass as bass
import concourse.mybir as mybir
import concourse.tile as tile
from concourse import bacc
from concourse._compat import with_exitstack
from concourse.bass_utils import run_bass_kernel_spmd

BF16 = mybir.dt.bfloat16
F32 = mybir.dt.float32
NPBF16 = ml_dtypes.bfloat16

B, T, C, H = 4, 2048, 1024, 16
D = C // H              # 64
HPC = 8                 # heads per core
NPAIR = HPC // 2        # head pairs per core
NCORES = 8
CC = C // 128           # 8 contraction chunks
NQT = T // 512          # 4 query slabs of 512
NTT = T // 128          # 16 token tiles of 128
SCALE = 1.0 / np.sqrt(D)


@with_exitstack
def _attention_body(ctx: ExitStack, tc: tile.TileContext, t_in: dict, t_out):
    nc = tc.nc
    consts = ctx.enter_context(tc.tile_pool(name="consts", bufs=1))
    qkp = ctx.enter_context(tc.tile_pool(name="qkp", bufs=1))
    vp = ctx.enter_context(tc.tile_pool(name="vp", bufs=1))
    ytp = ctx.enter_context(tc.tile_pool(name="ytp", bufs=1))
    ynp = ctx.enter_context(tc.tile_pool(name="ynp", bufs=2))
    ep = ctx.enter_context(tc.tile_pool(name="ep", bufs=6))
    rp = ctx.enter_context(tc.tile_pool(name="rp", bufs=4))
    outp = ctx.enter_context(tc.tile_pool(name="outp", bufs=4))
    mm_ps = ctx.enter_context(tc.tile_pool(name="mm_ps", bufs=2, space="PSUM"))
    s_ps = ctx.enter_context(tc.tile_pool(name="s_ps", bufs=2, space="PSUM"))
    y_ps = ctx.enter_context(tc.tile_pool(name="y_ps", bufs=1, space="PSUM"))

    # ---- constants / inputs to SBUF ----
    # Fused multi-chunk tiles: one DMA instruction covers many [128, .] chunks
    # (HWDGE descriptor-gen costs 625ns per DMA instruction, so instruction
    # count -- not bytes -- dominates the startup critical path).
    # wv and x share one DRAM buffer (host-concatenated) so the very first
    # vproj matmul's BOTH operands arrive in a single DMA chain
    wvxt = consts.tile([128, CC, 512 + T], BF16, tag="wvx")
    wqkt = consts.tile([128, CC, 1024], BF16, tag="wqk")
    wpt = consts.tile([128, NPAIR, 1024], BF16, tag="wp")
    wv = [wvxt[:, c, 0:512] for c in range(CC)]
    xT = [wvxt[:, c, 512:512 + T] for c in range(CC)]
    wqk = [wqkt[:, c, :] for c in range(CC)]
    wp = [wpt[:, j, :] for j in range(NPAIR)]
    bqk = consts.tile([128, 8], F32, tag="bqk")
    bv_row = consts.tile([1, 512], F32, tag="bv_row")
    bv_bc = consts.tile([128, 512], F32, tag="bv_bc")
    trieye = consts.tile([128, 2, 128], BF16, tag="trieye")
    tri = trieye[:, 0, :]
    eye = trieye[:, 1, :]

    # DMA order follows consumption: wv + x slab 0 (unlocks vproj 0-3) in
    # growing pieces, tiny consts, wqk (qkproj slab 0), remaining x, wp.
    wvxd = t_in["wvx"].rearrange("(c p) n -> p c n", p=128)
    wqkd = t_in["wqkT"].rearrange("(c p) n -> p c n", p=128)
    wpd = t_in["wpT"].rearrange("(j p) n -> p j n", p=128)
    for lo, hi in ((0, 1), (1, 3), (3, 5), (5, 8)):
        nc.sync.dma_start(wvxt[:, lo:hi, 0:1024], wvxd[:, lo:hi, 0:1024])
    nc.sync.dma_start(bv_row[:], t_in["bv"][:])
    nc.sync.dma_start(bqk[:], t_in["bqk"][:])
    nc.sync.dma_start(trieye[:], t_in["trieye"][:].rearrange("p (i n) -> p i n", i=2))
    nc.sync.dma_start(wqkt[:, 0:4, :], wqkd[:, 0:4, :])
    nc.sync.dma_start(wqkt[:, 4:8, :], wqkd[:, 4:8, :])
    nc.sync.dma_start(wvxt[:, :, 1024:512 + T], wvxd[:, :, 1024:512 + T])
    nc.sync.dma_start(wpt[:], wpd[:])
    nc.gpsimd.partition_broadcast(bv_bc[:], bv_row[:])

    # p-state warmup: the PE runs 2x slow until 3us of continuous execution;
    # spin dummy matmuls from t=0 so the ramp burns on garbage, not on the
    # first projection groups
    warm = consts.tile([128, 512], BF16, tag="warm")
    nc.vector.memset(warm[:], 1.0)
    for w in range(5):
        wps = mm_ps.tile([128, 512], F32, tag="mm", name="ps_w")
        nc.tensor.matmul(wps[:], warm[:, 0:128], warm[:], start=True, stop=True)
        nc.tensor.matmul(wps[:], warm[:, 0:128], warm[:], start=True, stop=True)

    qk = [qkp.tile([128, T], BF16, tag=f"qk{j}", name=f"qk{j}") for j in range(CC)]
    v = [vp.tile([128, HPC, D + 1], BF16, tag=f"v{i}", name=f"v{i}") for i in range(NTT)]
    for i in range(NTT):
        nc.vector.memset(v[i][:, :, D:D + 1], 1.0)
    yt = [ytp.tile([128, T], BF16, tag=f"yt{j}", name=f"yt{j}") for j in range(NPAIR)]
    y_nat = {}

    def vproj_group(i):
        ps = mm_ps.tile([128, 512], F32, tag="mm", name="ps_v")
        for cc in range(CC):
            nc.tensor.matmul(
                ps[:],
                xT[cc][:, i * 128:(i + 1) * 128],
                wv[cc][:],
                start=(cc == 0),
                stop=(cc == CC - 1),
            )
        nc.vector.tensor_add(
            v[i][:, :, 0:D],
            ps[:].rearrange("p (h d) -> p h d", h=HPC),
            bv_bc[:].rearrange("p (h d) -> p h d", h=HPC),
        )

    def qkproj_group(oc, tt):
        ps = mm_ps.tile([128, 512], F32, tag="mm", name="ps_qk")
        for cc in range(CC):
            nc.tensor.matmul(
                ps[:],
                wqk[cc][:, oc * 128:(oc + 1) * 128],
                xT[cc][:, tt * 512:(tt + 1) * 512],
                start=(cc == 0),
                stop=(cc == CC - 1),
            )
        nc.vector.tensor_scalar_add(
            qk[oc][:, tt * 512:(tt + 1) * 512], ps[:], bqk[:, oc:oc + 1]
        )

    def geom(qt, kc):
        m = kc - 4 * qt  # >= 0 on diagonal chunks
        qoff = 128 * m if m > 0 else 0
        return m, qoff, 512 - qoff

    def scores_chunk(qt, hp, kc):
        """Scores + exp + mask for one (head-pair, key-chunk); returns e."""
        q0 = qt * 512
        m, qoff, nw = geom(qt, kc)
        k0 = kc * 128
        meng = nc.vector
        sps = s_ps.tile([128, 1024], F32, tag="sps", name="sps")
        nc.tensor.matmul(
            sps[:, 0:nw],
            qk[4 + hp][0:64, k0:k0 + 128],
            qk[hp][0:64, q0 + qoff:q0 + 512],
            start=True, stop=True,
        )
        nc.tensor.matmul(
            sps[:, 512:512 + nw],
            qk[4 + hp][64:128, k0:k0 + 128],
            qk[hp][64:128, q0 + qoff:q0 + 512],
            start=True, stop=True,
        )
        e = ep.tile([128, 1024], BF16, tag="e", name="e")
        nc.scalar.activation(
            e[:].rearrange("p (i n) -> p i n", i=2)[:, :, 0:nw],
            sps[:].rearrange("p (i n) -> p i n", i=2)[:, :, 0:nw],
            mybir.ActivationFunctionType.Exp,
            scale=float(SCALE),
        )
        if m >= 0:
            meng.tensor_mul(
                e[:].rearrange("p (i n) -> p i n", i=2)[:, :, 0:128],
                e[:].rearrange("p (i n) -> p i n", i=2)[:, :, 0:128],
                tri[:].unsqueeze(1).broadcast_to([128, 2, 128]),
            )
        return e

    def ev_chunk(qt, hp, kc, ya, yb, e):
        """E@v with e as the stationary operand: per (head, 128-query slice)
        accumulate [128 q, 65] over key chunks; N=65 per matmul.

        PSUM start=True lazily zeroes the whole 2KB bank, so exactly ONE
        start per tile (first write poisons the bank; the other regions'
        first writes land on pending-zero bytes) and ONE stop on the bank's
        final write."""
        m, qoff, _ = geom(qt, kc)
        for h2, ydst in ((0, ya), (1, yb)):
            for qs in range(max(m, 0), 4):
                c0 = h2 * 512 + qs * 128 - qoff
                nc.tensor.matmul(
                    ydst[:, qs, :],
                    e[:, c0:c0 + 128],
                    v[kc][:, 2 * hp + h2, :],
                    start=(kc == 0 and qs == 0),
                    stop=(kc == 4 * qt + 3 and qs == 3),
                )

    def norm_pair(qt, hp, ya, yb):
        """Batched normalize of a whole pair: reciprocal of the denominator
        columns + broadcast multiply, PSUM -> y_nat bf16."""
        for h2, ysrc in ((0, ya), (1, yb)):
            h = 2 * hp + h2
            r = rp.tile([128, 4], F32, tag="r", name="r")
            nc.vector.reciprocal(r[:], ysrc[:, :, D])
            nc.vector.tensor_mul(
                y_nat[qt][:, :, h * D:(h + 1) * D],
                ysrc[:, :, 0:D],
                r[:].unsqueeze(2).broadcast_to([128, 4, D]),
            )

    def norm_qs(qt, hp, qs, ya, yb, on_act=False):
        """Per-query-slice normalize (tail of the last pair). The very last
        slice multiplies on ACT (idle once the exp chain drains) via an
        activation Copy with a per-partition scale."""
        for h2, ysrc in ((0, ya), (1, yb)):
            h = 2 * hp + h2
            r = rp.tile([128, 1], F32, tag="r1", name="r1")
            nc.vector.reciprocal(r[:], ysrc[:, qs, D:D + 1])
            if on_act:
                nc.scalar.activation(
                    y_nat[qt][:, qs, h * D:(h + 1) * D],
                    ysrc[:, qs, 0:D],
                    mybir.ActivationFunctionType.Copy,
                    scale=r[:],
                )
            else:
                nc.vector.tensor_mul(
                    y_nat[qt][:, qs, h * D:(h + 1) * D],
                    ysrc[:, qs, 0:D],
                    r[:].broadcast_to([128, D]),
                )

    def tdma(qt, qs, j):
        """yt[j] gets the transposed 128x128 block via the DMA XBAR."""
        nc.sync.dma_start(
            yt[j][:, qt * 512 + qs * 128:qt * 512 + (qs + 1) * 128],
            y_nat[qt][:, qs, j * 128:(j + 1) * 128],
            transpose=True,
        )

    def t_pe(qt, qs, j, on_act=False):
        """PE transpose + copy: lower latency than the DMA XBAR path."""
        ps = mm_ps.tile([128, 1024], BF16, tag="mm", name="ps_t")
        nc.tensor.transpose(ps[:, 0:128], y_nat[qt][:, qs, j * 128:(j + 1) * 128],
                            eye[:])
        dst = yt[j][:, qt * 512 + qs * 128:qt * 512 + (qs + 1) * 128]
        if on_act:
            nc.scalar.activation(dst, ps[:, 0:128],
                                 mybir.ActivationFunctionType.Copy)
        else:
            nc.vector.tensor_copy(dst, ps[:, 0:128])

    obuf = {}
    partials = {}

    def outproj_prerun(i, oh):
        """Accumulate pairs 0..2 of the final slab's output projection while
        the last pair's exp chain still runs; only the j=3 matmul and a DVE
        add remain after the last exp."""
        ps = mm_ps.tile([128, 512], F32, tag="mm", name="ps_pr")
        for j in range(NPAIR - 1):
            nc.tensor.matmul(
                ps[:],
                yt[j][:, i * 128:(i + 1) * 128],
                wp[j][:, oh * 512:(oh + 1) * 512],
                start=(j == 0),
                stop=(j == NPAIR - 2),
            )
        if i not in partials:
            partials[i] = outp.tile([128, 1024], BF16, tag="pp", name=f"pp{i}",
                                    bufs=4)
        nc.vector.tensor_copy(partials[i][:, oh * 512:(oh + 1) * 512], ps[:])

    def outproj_tail(i):
        """j3 matmul + partial add + ship, for one 128-token tile. The final
        tile (emitted after the last scores chunk) borrows a free scores-psum
        slot so both halves land in one [128,1024] tile -- one add, one DMA;
        earlier tiles would steal the slot from the still-running exp
        pipeline, so they go per-half through the mm rotation."""
        if i not in obuf:
            obuf[i] = outp.tile([128, 1024], BF16, tag="ob", name=f"ob{i}")
        ob = obuf[i]
        for oh in range(2):
            ps = mm_ps.tile([128, 512], F32, tag="mm", name="ps_tl")
            nc.tensor.matmul(
                ps[:],
                yt[NPAIR - 1][:, i * 128:(i + 1) * 128],
                wp[NPAIR - 1][:, oh * 512:(oh + 1) * 512],
                start=True, stop=True,
            )
            dst = ob[:, oh * 512:(oh + 1) * 512]
            nc.vector.tensor_add(dst, ps[:],
                                 partials[i][:, oh * 512:(oh + 1) * 512])
            nc.sync.dma_start(
                t_out[i * 128:(i + 1) * 128, oh * 512:(oh + 1) * 512],
                dst,
            )

    def outproj_group(i, oh):
        ps = mm_ps.tile([128, 512], F32, tag="mm", name="ps_op")
        for j in range(NPAIR):
            nc.tensor.matmul(
                ps[:],
                yt[j][:, i * 128:(i + 1) * 128],
                wp[j][:, oh * 512:(oh + 1) * 512],
                start=(j == 0),
                stop=(j == NPAIR - 1),
            )
        if i not in obuf:
            obuf[i] = outp.tile([128, 1024], BF16, tag="ob", name=f"ob{i}")
        ob = obuf[i]
        nc.vector.tensor_copy(ob[:, oh * 512:(oh + 1) * 512], ps[:])
        if i >= 4 * (NQT - 1):
            # final slab: ship each half as soon as its copy lands
            nc.sync.dma_start(
                t_out[i * 128:(i + 1) * 128, oh * 512:(oh + 1) * 512],
                ob[:, oh * 512:(oh + 1) * 512],
            )
        elif oh == 1:
            nc.sync.dma_start(t_out[i * 128:(i + 1) * 128, :], ob[:])

    # ---- schedule ----
    # Fill units per slab, sized to the slab's exp-vs-PE deficit. Each item
    # is (due, fn): the unit MUST be emitted before chunk-step `due` of its
    # slab (PE executes in program order, so a consumer emitted before its
    # producer deadlocks); due=None means "any time, flush by slab end".
    # Dues: vproj(g) feeds EV at pair-0 step kc=g; qkproj(oc=hp, tt) feeds
    # the one-ahead scores lookahead at step hp*L-1 of slab tt; k-halves
    # (oc=4+hp) feed the lookahead of pair hp's first slab-tt key chunk.
    def L(qt):
        return 4 * qt + 4

    fills = {
        0: [(15 if oc == 0 else None, lambda oc=oc: qkproj_group(oc, 1))
            for oc in range(CC)],
        1: ([(g, lambda i=i: vproj_group(i)) for g, i in
             zip(range(4, 8), range(4, 8))]
            + [(31 if oc == 0 else None, lambda oc=oc: qkproj_group(oc, 2))
               for oc in range(CC)]),
        2: ([(g, lambda i=i: vproj_group(i)) for g, i in
             zip(range(8, 12), range(8, 12))]
            + [(None, lambda i=i, oh=oh: outproj_group(i, oh))
               for i in range(0, 8) for oh in range(2)]
            + [(47, lambda: qkproj_group(0, 3))]),
        3: ([(11, lambda: qkproj_group(4, 3))]
            + [(g, lambda i=i: vproj_group(i)) for g, i in
               zip(range(12, 16), range(12, 16))]
            + [(hp * 16 - 1, lambda oc=hp: qkproj_group(oc, 3))
               for hp in (1, 2, 3)]
            + [(hp * 16 + 11, lambda oc=hp: qkproj_group(4 + oc, 3))
               for hp in (1, 2, 3)]),
    }
    # Late fill for slab 3's back half, where the fill deficit (ACT-paced
    # chunks vs small diagonal matmuls) is otherwise uncovered: outproj(2)
    # (ready since slab-2 end) through pairs 2-3, and the outproj(3)
    # pre-runs once pair 2's transposes have landed (after step 47).
    late_fills = {qt: [] for qt in range(NQT)}
    late_fills[3] = sorted(
        [(34 + 2 * k, fn) for k, fn in enumerate(
            lambda i=i, oh=oh: outproj_group(i, oh)
            for i in range(8, 12) for oh in range(2))],
        key=lambda p: p[0],
    )

    # Global chunk sequence; scores are pipelined one chunk ahead across pair
    # AND slab boundaries so the PE always has the next scores queued while
    # ACT digests the exp backlog.
    chunks = [(qt, hp, kc)
              for qt in range(NQT)
              for hp in range(NPAIR)
              for kc in range(4 * qt + 4)]
    nxt = {chunks[n]: chunks[n + 1] for n in range(len(chunks) - 1)}

    state = {}

    def chunk_step(qt, hp, kc):
        if not state:
            state["e"] = scores_chunk(qt, hp, kc)
        e = state["e"]
        if (qt, hp, kc) in nxt:
            state["e"] = scores_chunk(*nxt[(qt, hp, kc)])
        if kc == 0:
            state["y"] = (
                y_ps.tile([128, 4, D + 1], F32, tag="ya", name="ya", bufs=1),
                y_ps.tile([128, 4, D + 1], F32, tag="yb", name="yb", bufs=1),
            )
        ev_chunk(qt, hp, kc, state["y"][0], state["y"][1], e)

    def norm_qs(qt, hp, qs, ya, yb, on_act=False):
        """Per-query-slice normalize (tail of the last pair). The final
        slice multiplies on ACT (idle once the exp chain drains) via an
        activation Copy with a per-partition scale."""
        for h2, ysrc in ((0, ya), (1, yb)):
            h = 2 * hp + h2
            r = rp.tile([128, 1], F32, tag="r1", name="r1", bufs=8)
            nc.vector.reciprocal(r[:], ysrc[:, qs, D:D + 1])
            if on_act:
                nc.scalar.activation(
                    y_nat[qt][:, qs, h * D:(h + 1) * D],
                    ysrc[:, qs, 0:D],
                    mybir.ActivationFunctionType.Copy,
                    scale=r[:],
                )
            else:
                nc.vector.tensor_mul(
                    y_nat[qt][:, qs, h * D:(h + 1) * D],
                    ysrc[:, qs, 0:D],
                    r[:].broadcast_to([128, D]),
                )

    def post_chunk(qt, hp, kc):
        """Normalize/transpose work to emit right after a chunk. The last
        pair of the last slab staggers its per-slice tail: normalize at its
        diagonal chunk, transpose one chunk later, j3+add two chunks later,
        so each cross-engine chain's latency hides behind the next chunks
        instead of head-of-line blocking the in-order queues."""
        last_pair = (qt == NQT - 1 and hp == NPAIR - 1)
        if last_pair:
            qs = kc - 4 * qt
            if qs == -1:
                for i in range(12, 16):
                    for oh in range(2):
                        outproj_prerun(i, oh)
            if qs < 0:
                return
            norm_qs(qt, hp, qs, *state["y"], on_act=(qs == 3))
            t_pe(qt, qs, hp, on_act=(qs == 3))
            outproj_tail(4 * qt + qs)
        elif kc == 4 * qt + 3:
            if qt not in y_nat:
                y_nat[qt] = ynp.tile([128, 4, 512], BF16, tag="yn",
                                     name=f"yn{qt}")
            norm_pair(qt, hp, *state["y"])
            for qs in range(4):
                tdma(qt, qs, hp)

    # prologue: vproj for slab-0 tokens, then qkproj slab 0
    for i in range(4):
        vproj_group(i)
    for oc in range(CC):
        qkproj_group(oc, 0)

    # main loop: per slab, interleave attention chunks with the fill list.
    # Forced emission for due items; otherwise paced evenly across the slab.
    for qt in range(NQT):
        steps = [(hp, kc) for hp in range(NPAIR) for kc in range(4 * qt + 4)]
        nstep = len(steps)
        fill = list(fills[qt])
        late = list(late_fills[qt])
        span_e = max(1, int(nstep * (0.62 if late else 1.0)))
        total = len(fill)
        emitted = 0
        for n, (hp, kc) in enumerate(steps):
            # forced: everything that must precede this chunk step
            keep = []
            for due, fn in fill:
                if due is not None and due <= n:
                    fn()
                    emitted += 1
                else:
                    keep.append((due, fn))
            fill = keep
            chunk_step(qt, hp, kc)
            post_chunk(qt, hp, kc)
            # paced: early list across the first span_e steps
            target = min(total, (total * (n + 1) + span_e - 1) // span_e)
            while emitted < target and fill:
                due, fn = fill.pop(0)
                fn()
                emitted += 1
            # late list: each item fires at its start step
            while late and late[0][0] <= n:
                late.pop(0)[1]()
        for due, fn in fill + late:
            fn()

    if t_in.get("dbg"):
        for j in range(NPAIR):
            nc.sync.dma_start(t_in["dbg_yt"][j * 128:(j + 1) * 128, :], yt[j][:])
        for qt in range(NQT):
            nc.sync.dma_start(
                t_in["dbg_yn"][qt * 128:(qt + 1) * 128, :],
                y_nat[qt][:].rearrange("p a b -> p (a b)"),
            )


def build_model():
    nc = bacc.Bacc(
        "TRN2",
        target_bir_lowering=False,
        debug=False,
        enable_asserts=False,
        num_devices=NCORES,
    )
    t_in = {
        "wvx": nc.dram_tensor("wvx", [C, 512 + T], BF16, kind="ExternalInput").ap(),
        "wqkT": nc.dram_tensor("wqkT", [C, 1024], BF16, kind="ExternalInput").ap(),
        "wpT": nc.dram_tensor("wpT", [512, C], BF16, kind="ExternalInput").ap(),
        "bqk": nc.dram_tensor("bqk", [128, 8], F32, kind="ExternalInput").ap(),
        "bv": nc.dram_tensor("bv", [1, 512], F32, kind="ExternalInput").ap(),
        "trieye": nc.dram_tensor("trieye", [128, 256], BF16, kind="ExternalInput").ap(),
    }
    t_out = nc.dram_tensor("out", [T, C], BF16, kind="ExternalOutput").ap()
    with tile.TileContext(nc) as tc:
        _attention_body(tc, t_in, t_out)
    nc.compile()
    return nc


def make_in_maps(x, w_attn, b_attn, w_proj):
    """Host-side sharding: per-core input dict for core (b, hg)."""
    trieye = np.concatenate(
        [np.triu(np.ones((128, 128), np.float32)), np.eye(128, dtype=np.float32)],
        axis=1,
    ).astype(NPBF16)
    in_maps = []
    xT_cache = {}
    for cid in range(NCORES):
        b, hg = cid // 2, cid % 2
        h0 = hg * HPC
        if b not in xT_cache:
            xT_cache[b] = np.ascontiguousarray(x[b].T).astype(NPBF16)
        rq = slice(h0 * D, (h0 + HPC) * D)
        rk = slice(C + h0 * D, C + (h0 + HPC) * D)
        rv = slice(2 * C + h0 * D, 2 * C + (h0 + HPC) * D)
        wqkT = np.ascontiguousarray(
            np.concatenate([w_attn[rq], w_attn[rk]], axis=0).T
        ).astype(NPBF16)
        wvT = np.ascontiguousarray(w_attn[rv].T).astype(NPBF16)
        wpT = np.ascontiguousarray(w_proj[:, h0 * D:(h0 + HPC) * D].T).astype(NPBF16)
        bqk = np.stack(
            [b_attn[rq].reshape(4, 128)[j] for j in range(4)]
            + [b_attn[rk].reshape(4, 128)[j] for j in range(4)],
            axis=1,
        ).astype(np.float32)
        bv = b_attn[rv].reshape(1, 512).astype(np.float32)
        in_maps.append({
            "wvx": np.ascontiguousarray(
                np.concatenate([wvT, xT_cache[b]], axis=1)),
            "wqkT": wqkT,
            "wpT": wpT,
            "bqk": np.ascontiguousarray(bqk),
            "bv": bv,
            "trieye": trieye,
        })
    return in_maps


_NC_CACHE = []


def kernel(x, w_attn, b_attn, w_proj, b_proj):
    x = np.asarray(x, dtype=np.float32)
    w_attn = np.asarray(w_attn, dtype=np.float32)
    b_attn = np.asarray(b_attn, dtype=np.float32)
    w_proj = np.asarray(w_proj, dtype=np.float32)
    b_proj = np.asarray(b_proj, dtype=np.float32)

    if not _NC_CACHE:
        _NC_CACHE.append(build_model())
    nc = _NC_CACHE[0]
    in_maps = make_in_maps(x, w_attn, b_attn, w_proj)
    res = None
    for attempt in range(3):
        try:
            res = run_bass_kernel_spmd(nc, in_maps, core_ids=list(range(NCORES)))
            break
        except Exception:
            if attempt == 2:
                raise
            import time
            time.sleep(5)
    out = np.empty((B, T, C), np.float32)
    for b in range(B):
        out[b] = (res.results[2 * b]["out"].astype(np.float32)
                  + res.results[2 * b + 1]["out"].astype(np.float32))
    out += b_proj[None, None, :]
    return out


# revision 68
# speedup vs baseline: 1.0055x; 1.0013x over previous
"""Causal self-attention (B=4, T=2048, C=1024, H=16) on 8 trn2 NeuronCores.

Sharding: core = (batch b, head-group hg) -> 4 x 2 grid. Each core computes
attention for 8 of the 16 heads of one batch plus the partial output
projection over its heads' columns; the host sums the two partials per batch
and adds b_proj.

Device-side layout (PE engine cost = streamed-N only; K/M are free):
  - host supplies x^T [C, T] and W^T slices in bf16
  - q,k produced transposed [d, t]; v natural [t, d] with a ones column
    (M = 65) that accumulates the softmax denominator
  - scores S^T [tk, tq]: lhsT = k (K=d=64), rhs = q, N = queries. This is
    N-optimal (score elements / 128).
  - exp on ACT (scale fused); causal diagonal chunks get narrowed matmuls
    plus one [128,128] triangular mask multiply
  - E@v uses the e-slices as the STATIONARY operand: out y [128 queries, 65]
    accumulates over key chunks with N=65 per step (vs N=512 in the naive
    orientation) -- halves attention-phase PE time
  - normalize: DVE reciprocal of the denominator column + broadcast multiply
    straight out of PSUM into y_nat bf16
  - y_nat [q, c] -> yt [c, q] via DMA-transpose (XBAR, 112ns/128x128 tile);
    the tail-critical blocks of the last pair use a PE transpose instead to
    dodge the ~1.7us DMA semaphore latency
  - output projection accumulates yt @ wp per 128-token tile; partials are
    shipped as bf16 (the host sums the two partials per batch in f32)
  - schedule: projection / output-projection groups are spliced between
    attention chunks per-slab to cover the exp (ACT) serial backlog; the
    last slab gets qkproj(3)+vproj(12..15)+outproj(2); the last pair of the
    last slab normalizes per-128-query-slice so outproj(3) can start before
    the final chunk retires.
"""

import sys

if "/opt/trn_rl_repo" not in sys.path:
    sys.path.insert(0, "/opt/trn_rl_repo")

from contextlib import ExitStack

import ml_dtypes
import numpy as np

import concourse.bass as bass
import concourse.mybir as mybir
import concourse.tile as tile
from concourse import bacc
from concourse._compat import with_exitstack
from concourse.bass_utils import run_bass_kernel_spmd

BF16 = mybir.dt.bfloat16
F32 = mybir.dt.float32
NPBF16 = ml_dtypes.bfloat16

B, T, C, H = 4, 2048, 1024, 16
D = C // H              # 64
HPC = 8                 # heads per core
NPAIR = HPC // 2        # head pairs per core
NCORES = 8
CC = C // 128           # 8 contraction chunks
NQT = T // 512          # 4 query slabs of 512
NTT = T // 128          # 16 token tiles of 128
SCALE = 1.0 / np.sqrt(D)


@with_exitstack
def _attention_body(ctx: ExitStack, tc: tile.TileContext, t_in: dict, t_out):
    nc = tc.nc
    consts = ctx.enter_context(tc.tile_pool(name="consts", bufs=1))
    qkp = ctx.enter_context(tc.tile_pool(name="qkp", bufs=1))
    vp = ctx.enter_context(tc.tile_pool(name="vp", bufs=1))
    ytp = ctx.enter_context(tc.tile_pool(name="ytp", bufs=1))
    ynp = ctx.enter_context(tc.tile_pool(name="ynp", bufs=2))
    ep = ctx.enter_context(tc.tile_pool(name="ep", bufs=6))
    rp = ctx.enter_context(tc.tile_pool(name="rp", bufs=4))
    outp = ctx.enter_context(tc.tile_pool(name="outp", bufs=4))
    mm_ps = ctx.enter_context(tc.tile_pool(name="mm_ps", bufs=2, space="PSUM"))
    s_ps = ctx.enter_context(tc.tile_pool(name="s_ps", bufs=2, space="PSUM"))
    y_ps = ctx.enter_context(tc.tile_pool(name="y_ps", bufs=1, space="PSUM"))

    # ---- constants / inputs to SBUF ----
    # Fused multi-chunk tiles: one DMA instruction covers many [128, .] chunks
    # (HWDGE descriptor-gen costs 625ns per DMA instruction, so instruction
    # count -- not bytes -- dominates the startup critical path).
    # wv and x share one DRAM buffer (host-concatenated) so the very first
    # vproj matmul's BOTH operands arrive in a single DMA chain
    wvxt = consts.tile([128, CC, 512 + T], BF16, tag="wvx")
    wqkt = consts.tile([128, CC, 1024], BF16, tag="wqk")
    wpt = consts.tile([128, NPAIR, 1024], BF16, tag="wp")
    wv = [wvxt[:, c, 0:512] for c in range(CC)]
    xT = [wvxt[:, c, 512:512 + T] for c in range(CC)]
    wqk = [wqkt[:, c, :] for c in range(CC)]
    wp = [wpt[:, j, :] for j in range(NPAIR)]
    bqk = consts.tile([128, 8], F32, tag="bqk")
    bv_row = consts.tile([1, 512], F32, tag="bv_row")
    bv_bc = consts.tile([128, 512], F32, tag="bv_bc")
    trieye = consts.tile([128, 2, 128], BF16, tag="trieye")
    tri = trieye[:, 0, :]
    eye = trieye[:, 1, :]

    # DMA order follows consumption: wv + x slab 0 (unlocks vproj 0-3) in
    # growing pieces, tiny consts, wqk (qkproj slab 0), remaining x, wp.
    wvxd = t_in["wvx"].rearrange("(c p) n -> p c n", p=128)
    wqkd = t_in["wqkT"].rearrange("(c p) n -> p c n", p=128)
    wpd = t_in["wpT"].rearrange("(j p) n -> p j n", p=128)
    for lo, hi in ((0, 1), (1, 3), (3, 5), (5, 8)):
        nc.sync.dma_start(wvxt[:, lo:hi, 0:1024], wvxd[:, lo:hi, 0:1024])
    nc.sync.dma_start(bv_row[:], t_in["bv"][:])
    nc.sync.dma_start(bqk[:], t_in["bqk"][:])
    nc.sync.dma_start(trieye[:], t_in["trieye"][:].rearrange("p (i n) -> p i n", i=2))
    nc.sync.dma_start(wqkt[:, 0:4, :], wqkd[:, 0:4, :])
    nc.sync.dma_start(wqkt[:, 4:8, :], wqkd[:, 4:8, :])
    nc.sync.dma_start(wvxt[:, :, 1024:512 + T], wvxd[:, :, 1024:512 + T])
    nc.sync.dma_start(wpt[:], wpd[:])
    nc.gpsimd.partition_broadcast(bv_bc[:], bv_row[:])

    # p-state warmup: the PE runs 2x slow until 3us of continuous execution;
    # spin dummy matmuls from t=0 so the ramp burns on garbage, not on the
    # first projection groups
    warm = consts.tile([128, 512], BF16, tag="warm")
    nc.vector.memset(warm[:], 1.0)
    for w in range(5):
        wps = mm_ps.tile([128, 512], F32, tag="mm", name="ps_w")
        nc.tensor.matmul(wps[:], warm[:, 0:128], warm[:], start=True, stop=True)
        nc.tensor.matmul(wps[:], warm[:, 0:128], warm[:], start=True, stop=True)

    qk = [qkp.tile([128, T], BF16, tag=f"qk{j}", name=f"qk{j}") for j in range(CC)]
    v = [vp.tile([128, HPC, D + 1], BF16, tag=f"v{i}", name=f"v{i}") for i in range(NTT)]
    for i in range(NTT):
        nc.vector.memset(v[i][:, :, D:D + 1], 1.0)
    yt = [ytp.tile([128, T], BF16, tag=f"yt{j}", name=f"yt{j}") for j in range(NPAIR)]
    y_nat = {}

    def vproj_group(i):
        ps = mm_ps.tile([128, 512], F32, tag="mm", name="ps_v")
        for cc in range(CC):
            nc.tensor.matmul(
                ps[:],
                xT[cc][:, i * 128:(i + 1) * 128],
                wv[cc][:],
                start=(cc == 0),
                stop=(cc == CC - 1),
            )
        nc.vector.tensor_add(
            v[i][:, :, 0:D],
            ps[:].rearrange("p (h d) -> p h d", h=HPC),
            bv_bc[:].rearrange("p (h d) -> p h d", h=HPC),
        )

    def qkproj_group(oc, tt):
        ps = mm_ps.tile([128, 512], F32, tag="mm", name="ps_qk")
        for cc in range(CC):
            nc.tensor.matmul(
                ps[:],
                wqk[cc][:, oc * 128:(oc + 1) * 128],
                xT[cc][:, tt * 512:(tt + 1) * 512],
                start=(cc == 0),
                stop=(cc == CC - 1),
            )
        nc.vector.tensor_scalar_add(
            qk[oc][:, tt * 512:(tt + 1) * 512], ps[:], bqk[:, oc:oc + 1]
        )

    def geom(qt, kc):
        m = kc - 4 * qt  # >= 0 on diagonal chunks
        qoff = 128 * m if m > 0 else 0
        return m, qoff, 512 - qoff

    def scores_chunk(qt, hp, kc):
        """Scores + exp + mask for one (head-pair, key-chunk); returns e."""
        q0 = qt * 512
        m, qoff, nw = geom(qt, kc)
        k0 = kc * 128
        # the last pair's diagonal masks ride gpsimd (SBUF-only, legal):
        # they would otherwise queue on DVE ahead of the tail's norm chains
        meng = (nc.gpsimd if (qt == NQT - 1 and hp == NPAIR - 1 and
                              kc - 4 * qt >= 0) else nc.vector)
        sps = s_ps.tile([128, 1024], F32, tag="sps", name="sps")
        nc.tensor.matmul(
            sps[:, 0:nw],
            qk[4 + hp][0:64, k0:k0 + 128],
            qk[hp][0:64, q0 + qoff:q0 + 512],
            start=True, stop=True,
        )
        nc.tensor.matmul(
            sps[:, 512:512 + nw],
            qk[4 + hp][64:128, k0:k0 + 128],
            qk[hp][64:128, q0 + qoff:q0 + 512],
            start=True, stop=True,
        )
        e = ep.tile([128, 1024], BF16, tag="e", name="e")
        nc.scalar.activation(
            e[:].rearrange("p (i n) -> p i n", i=2)[:, :, 0:nw],
            sps[:].rearrange("p (i n) -> p i n", i=2)[:, :, 0:nw],
            mybir.ActivationFunctionType.Exp,
            scale=float(SCALE),
        )
        if m >= 0:
            meng.tensor_mul(
                e[:].rearrange("p (i n) -> p i n", i=2)[:, :, 0:128],
                e[:].rearrange("p (i n) -> p i n", i=2)[:, :, 0:128],
                tri[:].unsqueeze(1).broadcast_to([128, 2, 128]),
            )
        return e

    def ev_chunk(qt, hp, kc, ya, yb, e):
        """E@v with e as the stationary operand: per (head, 128-query slice)
        accumulate [128 q, 65] over key chunks; N=65 per matmul.

        PSUM start=True lazily zeroes the whole 2KB bank, so exactly ONE
        start per tile (first write poisons the bank; the other regions'
        first writes land on pending-zero bytes) and ONE stop on the bank's
        final write."""
        m, qoff, _ = geom(qt, kc)
        for h2, ydst in ((0, ya), (1, yb)):
            for qs in range(max(m, 0), 4):
                c0 = h2 * 512 + qs * 128 - qoff
                nc.tensor.matmul(
                    ydst[:, qs, :],
                    e[:, c0:c0 + 128],
                    v[kc][:, 2 * hp + h2, :],
                    start=(kc == 0 and qs == 0),
                    stop=(kc == 4 * qt + 3 and qs == 3),
                )

    def norm_pair(qt, hp, ya, yb):
        """Batched normalize of a whole pair: reciprocal of the denominator
        columns + broadcast multiply, PSUM -> y_nat bf16."""
        for h2, ysrc in ((0, ya), (1, yb)):
            h = 2 * hp + h2
            r = rp.tile([128, 4], F32, tag="r", name="r")
            nc.vector.reciprocal(r[:], ysrc[:, :, D])
            nc.vector.tensor_mul(
                y_nat[qt][:, :, h * D:(h + 1) * D],
                ysrc[:, :, 0:D],
                r[:].unsqueeze(2).broadcast_to([128, 4, D]),
            )

    def norm_qs(qt, hp, qs, ya, yb, on_act=False):
        """Per-query-slice normalize (tail of the last pair). The very last
        slice multiplies on ACT (idle once the exp chain drains) via an
        activation Copy with a per-partition scale."""
        for h2, ysrc in ((0, ya), (1, yb)):
            h = 2 * hp + h2
            r = rp.tile([128, 1], F32, tag="r1", name="r1")
            nc.vector.reciprocal(r[:], ysrc[:, qs, D:D + 1])
            if on_act:
                nc.scalar.activation(
                    y_nat[qt][:, qs, h * D:(h + 1) * D],
                    ysrc[:, qs, 0:D],
                    mybir.ActivationFunctionType.Copy,
                    scale=r[:],
                )
            else:
                nc.vector.tensor_mul(
                    y_nat[qt][:, qs, h * D:(h + 1) * D],
                    ysrc[:, qs, 0:D],
                    r[:].broadcast_to([128, D]),
                )

    def tdma(qt, qs, j):
        """yt[j] gets the transposed 128x128 block via the DMA XBAR."""
        nc.sync.dma_start(
            yt[j][:, qt * 512 + qs * 128:qt * 512 + (qs + 1) * 128],
            y_nat[qt][:, qs, j * 128:(j + 1) * 128],
            transpose=True,
        )

    def t_pe(qt, qs, j, on_act=False):
        """PE transpose + copy: lower latency than the DMA XBAR path."""
        ps = mm_ps.tile([128, 1024], BF16, tag="mm", name="ps_t")
        nc.tensor.transpose(ps[:, 0:128], y_nat[qt][:, qs, j * 128:(j + 1) * 128],
                            eye[:])
        dst = yt[j][:, qt * 512 + qs * 128:qt * 512 + (qs + 1) * 128]
        if on_act:
            nc.scalar.activation(dst, ps[:, 0:128],
                                 mybir.ActivationFunctionType.Copy)
        else:
            nc.vector.tensor_copy(dst, ps[:, 0:128])

    obuf = {}
    partials = {}

    def outproj_prerun(i, oh):
        """Accumulate pairs 0..2 of the final slab's output projection while
        the last pair's exp chain still runs; only the j=3 matmul and a DVE
        add remain after the last exp."""
        ps = mm_ps.tile([128, 512], F32, tag="mm", name="ps_pr")
        for j in range(NPAIR - 1):
            nc.tensor.matmul(
                ps[:],
                yt[j][:, i * 128:(i + 1) * 128],
                wp[j][:, oh * 512:(oh + 1) * 512],
                start=(j == 0),
                stop=(j == NPAIR - 2),
            )
        if i not in partials:
            partials[i] = outp.tile([128, 1024], BF16, tag="pp", name=f"pp{i}",
                                    bufs=4)
        nc.vector.tensor_copy(partials[i][:, oh * 512:(oh + 1) * 512], ps[:])

    def outproj_tail(i):
        """j3 matmul + partial add + ship, for one 128-token tile. The final
        tile (emitted after the last scores chunk) borrows a free scores-psum
        slot so both halves land in one [128,1024] tile -- one add, one DMA;
        earlier tiles would steal the slot from the still-running exp
        pipeline, so they go per-half through the mm rotation."""
        if i not in obuf:
            obuf[i] = outp.tile([128, 1024], BF16, tag="ob", name=f"ob{i}")
        ob = obuf[i]
        for oh in range(2):
            ps = mm_ps.tile([128, 512], F32, tag="mm", name="ps_tl")
            nc.tensor.matmul(
                ps[:],
                yt[NPAIR - 1][:, i * 128:(i + 1) * 128],
                wp[NPAIR - 1][:, oh * 512:(oh + 1) * 512],
                start=True, stop=True,
            )
            dst = ob[:, oh * 512:(oh + 1) * 512]
            nc.vector.tensor_add(dst, ps[:],
                                 partials[i][:, oh * 512:(oh + 1) * 512])
            nc.sync.dma_start(
                t_out[i * 128:(i + 1) * 128, oh * 512:(oh + 1) * 512],
                dst,
            )

    def outproj_group(i, oh):
        ps = mm_ps.tile([128, 512], F32, tag="mm", name="ps_op")
        for j in range(NPAIR):
            nc.tensor.matmul(
                ps[:],
                yt[j][:, i * 128:(i + 1) * 128],
                wp[j][:, oh * 512:(oh + 1) * 512],
                start=(j == 0),
                stop=(j == NPAIR - 1),
            )
        if i not in obuf:
            obuf[i] = outp.tile([128, 1024], BF16, tag="ob", name=f"ob{i}")
        ob = obuf[i]
        nc.vector.tensor_copy(ob[:, oh * 512:(oh + 1) * 512], ps[:])
        if i >= 4 * (NQT - 1):
            # final slab: ship each half as soon as its copy lands
            nc.sync.dma_start(
                t_out[i * 128:(i + 1) * 128, oh * 512:(oh + 1) * 512],
                ob[:, oh * 512:(oh + 1) * 512],
            )
        elif oh == 1:
            nc.sync.dma_start(t_out[i * 128:(i + 1) * 128, :], ob[:])

    # ---- schedule ----
    # Fill units per slab, sized to the slab's exp-vs-PE deficit. Each item
    # is (due, fn): the unit MUST be emitted before chunk-step `due` of its
    # slab (PE executes in program order, so a consumer emitted before its
    # producer deadlocks); due=None means "any time, flush by slab end".
    # Dues: vproj(g) feeds EV at pair-0 step kc=g; qkproj(oc=hp, tt) feeds
    # the one-ahead scores lookahead at step hp*L-1 of slab tt; k-halves
    # (oc=4+hp) feed the lookahead of pair hp's first slab-tt key chunk.
    def L(qt):
        return 4 * qt + 4

    fills = {
        0: [(15 if oc == 0 else None, lambda oc=oc: qkproj_group(oc, 1))
            for oc in range(CC)],
        1: ([(g, lambda i=i: vproj_group(i)) for g, i in
             zip(range(4, 8), range(4, 8))]
            + [(31 if oc == 0 else None, lambda oc=oc: qkproj_group(oc, 2))
               for oc in range(CC)]),
        2: ([(g, lambda i=i: vproj_group(i)) for g, i in
             zip(range(8, 12), range(8, 12))]
            + [(None, lambda i=i, oh=oh: outproj_group(i, oh))
               for i in range(0, 8) for oh in range(2)]
            + [(47, lambda: qkproj_group(0, 3))]),
        3: ([(11, lambda: qkproj_group(4, 3))]
            + [(g, lambda i=i: vproj_group(i)) for g, i in
               zip(range(12, 16), range(12, 16))]
            + [(hp * 16 - 1, lambda oc=hp: qkproj_group(oc, 3))
               for hp in (1, 2, 3)]
            + [(hp * 16 + 11, lambda oc=hp: qkproj_group(4 + oc, 3))
               for hp in (1, 2, 3)]),
    }
    # Late fill for slab 3's back half, where the fill deficit (ACT-paced
    # chunks vs small diagonal matmuls) is otherwise uncovered: outproj(2)
    # (ready since slab-2 end) through pairs 2-3, and the outproj(3)
    # pre-runs once pair 2's transposes have landed (after step 47).
    late_fills = {qt: [] for qt in range(NQT)}
    late_fills[3] = sorted(
        [(34 + 2 * k, fn) for k, fn in enumerate(
            lambda i=i, oh=oh: outproj_group(i, oh)
            for i in range(8, 12) for oh in range(2))],
        key=lambda p: p[0],
    )

    # Global chunk sequence; scores are pipelined one chunk ahead across pair
    # AND slab boundaries so the PE always has the next scores queued while
    # ACT digests the exp backlog.
    chunks = [(qt, hp, kc)
              for qt in range(NQT)
              for hp in range(NPAIR)
              for kc in range(4 * qt + 4)]
    nxt = {chunks[n]: chunks[n + 1] for n in range(len(chunks) - 1)}

    state = {}

    def chunk_step(qt, hp, kc):
        if not state:
            state["e"] = scores_chunk(qt, hp, kc)
        e = state["e"]
        if (qt, hp, kc) in nxt:
            state["e"] = scores_chunk(*nxt[(qt, hp, kc)])
        if kc == 0:
            state["y"] = (
                y_ps.tile([128, 4, D + 1], F32, tag="ya", name="ya", bufs=1),
                y_ps.tile([128, 4, D + 1], F32, tag="yb", name="yb", bufs=1),
            )
        ev_chunk(qt, hp, kc, state["y"][0], state["y"][1], e)

    def norm_qs(qt, hp, qs, ya, yb, on_act=False):
        """Per-query-slice normalize (tail of the last pair). The final
        slice multiplies on ACT (idle once the exp chain drains) via an
        activation Copy with a per-partition scale."""
        for h2, ysrc in ((0, ya), (1, yb)):
            h = 2 * hp + h2
            r = rp.tile([128, 1], F32, tag="r1", name="r1", bufs=8)
            nc.vector.reciprocal(r[:], ysrc[:, qs, D:D + 1])
            if on_act:
                nc.scalar.activation(
                    y_nat[qt][:, qs, h * D:(h + 1) * D],
                    ysrc[:, qs, 0:D],
                    mybir.ActivationFunctionType.Copy,
                    scale=r[:],
                )
            else:
                nc.vector.tensor_mul(
                    y_nat[qt][:, qs, h * D:(h + 1) * D],
                    ysrc[:, qs, 0:D],
                    r[:].broadcast_to([128, D]),
                )

    def post_chunk(qt, hp, kc):
        """Normalize/transpose work to emit right after a chunk. The last
        pair of the last slab staggers its per-slice tail: normalize at its
        diagonal chunk, transpose one chunk later, j3+add two chunks later,
        so each cross-engine chain's latency hides behind the next chunks
        instead of head-of-line blocking the in-order queues."""
        last_pair = (qt == NQT - 1 and hp == NPAIR - 1)
        if last_pair:
            qs = kc - 4 * qt
            if qs == -1:
                for i in range(12, 16):
                    for oh in range(2):
                        outproj_prerun(i, oh)
            if qs < 0:
                return
            norm_qs(qt, hp, qs, *state["y"], on_act=(qs == 3))
            t_pe(qt, qs, hp, on_act=(qs == 3))
            outproj_tail(4 * qt + qs)
        elif kc == 4 * qt + 3:
            if qt not in y_nat:
                y_nat[qt] = ynp.tile([128, 4, 512], BF16, tag="yn",
                                     name=f"yn{qt}")
            norm_pair(qt, hp, *state["y"])
            for qs in range(4):
                tdma(qt, qs, hp)

    # prologue: vproj for slab-0 tokens, then qkproj slab 0
    for i in range(4):
        vproj_group(i)
    for oc in range(CC):
        qkproj_group(oc, 0)

    # main loop: per slab, interleave attention chunks with the fill list.
    # Forced emission for due items; otherwise paced evenly across the slab.
    for qt in range(NQT):
        steps = [(hp, kc) for hp in range(NPAIR) for kc in range(4 * qt + 4)]
        nstep = len(steps)
        fill = list(fills[qt])
        late = list(late_fills[qt])
        span_e = max(1, int(nstep * (0.62 if late else 1.0)))
        total = len(fill)
        emitted = 0
        for n, (hp, kc) in enumerate(steps):
            # forced: everything that must precede this chunk step
            keep = []
            for due, fn in fill:
                if due is not None and due <= n:
                    fn()
                    emitted += 1
                else:
                    keep.append((due, fn))
            fill = keep
            chunk_step(qt, hp, kc)
            post_chunk(qt, hp, kc)
            # paced: early list across the first span_e steps
            target = min(total, (total * (n + 1) + span_e - 1) // span_e)
            while emitted < target and fill:
                due, fn = fill.pop(0)
                fn()
                emitted += 1
            # late list: each item fires at its start step
            while late and late[0][0] <= n:
                late.pop(0)[1]()
        for due, fn in fill + late:
            fn()

    if t_in.get("dbg"):
        for j in range(NPAIR):
            nc.sync.dma_start(t_in["dbg_yt"][j * 128:(j + 1) * 128, :], yt[j][:])
        for qt in range(NQT):
            nc.sync.dma_start(
                t_in["dbg_yn"][qt * 128:(qt + 1) * 128, :],
                y_nat[qt][:].rearrange("p a b -> p (a b)"),
            )


def build_model():
    nc = bacc.Bacc(
        "TRN2",
        target_bir_lowering=False,
        debug=False,
        enable_asserts=False,
        num_devices=NCORES,
    )
    t_in = {
        "wvx": nc.dram_tensor("wvx", [C, 512 + T], BF16, kind="ExternalInput").ap(),
        "wqkT": nc.dram_tensor("wqkT", [C, 1024], BF16, kind="ExternalInput").ap(),
        "wpT": nc.dram_tensor("wpT", [512, C], BF16, kind="ExternalInput").ap(),
        "bqk": nc.dram_tensor("bqk", [128, 8], F32, kind="ExternalInput").ap(),
        "bv": nc.dram_tensor("bv", [1, 512], F32, kind="ExternalInput").ap(),
        "trieye": nc.dram_tensor("trieye", [128, 256], BF16, kind="ExternalInput").ap(),
    }
    t_out = nc.dram_tensor("out", [T, C], BF16, kind="ExternalOutput").ap()
    with tile.TileContext(nc) as tc:
        _attention_body(tc, t_in, t_out)
    nc.compile()
    return nc


def make_in_maps(x, w_attn, b_attn, w_proj):
    """Host-side sharding: per-core input dict for core (b, hg)."""
    trieye = np.concatenate(
        [np.triu(np.ones((128, 128), np.float32)), np.eye(128, dtype=np.float32)],
        axis=1,
    ).astype(NPBF16)
    in_maps = []
    xT_cache = {}
    for cid in range(NCORES):
        b, hg = cid // 2, cid % 2
        h0 = hg * HPC
        if b not in xT_cache:
            xT_cache[b] = np.ascontiguousarray(x[b].T).astype(NPBF16)
        rq = slice(h0 * D, (h0 + HPC) * D)
        rk = slice(C + h0 * D, C + (h0 + HPC) * D)
        rv = slice(2 * C + h0 * D, 2 * C + (h0 + HPC) * D)
        wqkT = np.ascontiguousarray(
            np.concatenate([w_attn[rq], w_attn[rk]], axis=0).T
        ).astype(NPBF16)
        wvT = np.ascontiguousarray(w_attn[rv].T).astype(NPBF16)
        wpT = np.ascontiguousarray(w_proj[:, h0 * D:(h0 + HPC) * D].T).astype(NPBF16)
        bqk = np.stack(
            [b_attn[rq].reshape(4, 128)[j] for j in range(4)]
            + [b_attn[rk].reshape(4, 128)[j] for j in range(4)],
            axis=1,
        ).astype(np.float32)
        bv = b_attn[rv].reshape(1, 512).astype(np.float32)
        in_maps.append({
            "wvx": np.ascontiguousarray(
                np.concatenate([wvT, xT_cache[b]], axis=1)),
            "wqkT": wqkT,
            "wpT": wpT,
            "bqk": np.ascontiguousarray(bqk),
            "bv": bv,
            "trieye": trieye,
        })
    return in_maps


_NC_CACHE = []


def kernel(x, w_attn, b_attn, w_proj, b_proj):
    x = np.asarray(x, dtype=np.float32)
    w_attn = np.asarray(w_attn, dtype=np.float32)
    b_attn = np.asarray(b_attn, dtype=np.float32)
    w_proj = np.asarray(w_proj, dtype=np.float32)
    b_proj = np.asarray(b_proj, dtype=np.float32)

    if not _NC_CACHE:
        _NC_CACHE.append(build_model())
    nc = _NC_CACHE[0]
    in_maps = make_in_maps(x, w_attn, b_attn, w_proj)
    res = None
    for attempt in range(3):
        try:
            res = run_bass_kernel_spmd(nc, in_maps, core_ids=list(range(NCORES)))
            break
        except Exception:
            if attempt == 2:
                raise
            import time
            time.sleep(5)
    out = np.empty((B, T, C), np.float32)
    for b in range(B):
        out[b] = (res.results[2 * b]["out"].astype(np.float32)
                  + res.results[2 * b + 1]["out"].astype(np.float32))
    out += b_proj[None, None, :]
    return out


# revision 73
# speedup vs baseline: 1.0060x; 1.0004x over previous
"""Causal self-attention (B=4, T=2048, C=1024, H=16) on 8 trn2 NeuronCores.

Sharding: core = (batch b, head-group hg) -> 4 x 2 grid. Each core computes
attention for 8 of the 16 heads of one batch plus the partial output
projection over its heads' columns; the host sums the two partials per batch
and adds b_proj.

Device-side layout (PE engine cost = streamed-N only; K/M are free):
  - host supplies x^T [C, T] and W^T slices in bf16
  - q,k produced transposed [d, t]; v natural [t, d] with a ones column
    (M = 65) that accumulates the softmax denominator
  - scores S^T [tk, tq]: lhsT = k (K=d=64), rhs = q, N = queries. This is
    N-optimal (score elements / 128).
  - exp on ACT (scale fused); causal diagonal chunks get narrowed matmuls
    plus one [128,128] triangular mask multiply
  - E@v uses the e-slices as the STATIONARY operand: out y [128 queries, 65]
    accumulates over key chunks with N=65 per step (vs N=512 in the naive
    orientation) -- halves attention-phase PE time
  - normalize: DVE reciprocal of the denominator column + broadcast multiply
    straight out of PSUM into y_nat bf16
  - y_nat [q, c] -> yt [c, q] via DMA-transpose (XBAR, 112ns/128x128 tile);
    the tail-critical blocks of the last pair use a PE transpose instead to
    dodge the ~1.7us DMA semaphore latency
  - output projection accumulates yt @ wp per 128-token tile; partials are
    shipped as bf16 (the host sums the two partials per batch in f32)
  - schedule: projection / output-projection groups are spliced between
    attention chunks per-slab to cover the exp (ACT) serial backlog; the
    last slab gets qkproj(3)+vproj(12..15)+outproj(2); the last pair of the
    last slab normalizes per-128-query-slice so outproj(3) can start before
    the final chunk retires.
"""

import sys

if "/opt/trn_rl_repo" not in sys.path:
    sys.path.insert(0, "/opt/trn_rl_repo")

from contextlib import ExitStack

import ml_dtypes
import numpy as np

import concourse.bass as bass
import concourse.mybir as mybir
import concourse.tile as tile
from concourse import bacc
from concourse._compat import with_exitstack
from concourse.bass_utils import run_bass_kernel_spmd

BF16 = mybir.dt.bfloat16
F32 = mybir.dt.float32
NPBF16 = ml_dtypes.bfloat16

B, T, C, H = 4, 2048, 1024, 16
D = C // H              # 64
HPC = 8                 # heads per core
NPAIR = HPC // 2        # head pairs per core
NCORES = 8
CC = C // 128           # 8 contraction chunks
NQT = T // 512          # 4 query slabs of 512
NTT = T // 128          # 16 token tiles of 128
SCALE = 1.0 / np.sqrt(D)


@with_exitstack
def _attention_body(ctx: ExitStack, tc: tile.TileContext, t_in: dict, t_out):
    nc = tc.nc
    consts = ctx.enter_context(tc.tile_pool(name="consts", bufs=1))
    qkp = ctx.enter_context(tc.tile_pool(name="qkp", bufs=1))
    vp = ctx.enter_context(tc.tile_pool(name="vp", bufs=1))
    ytp = ctx.enter_context(tc.tile_pool(name="ytp", bufs=1))
    ynp = ctx.enter_context(tc.tile_pool(name="ynp", bufs=2))
    ep = ctx.enter_context(tc.tile_pool(name="ep", bufs=6))
    rp = ctx.enter_context(tc.tile_pool(name="rp", bufs=4))
    outp = ctx.enter_context(tc.tile_pool(name="outp", bufs=4))
    mm_ps = ctx.enter_context(tc.tile_pool(name="mm_ps", bufs=2, space="PSUM"))
    s_ps = ctx.enter_context(tc.tile_pool(name="s_ps", bufs=2, space="PSUM"))
    y_ps = ctx.enter_context(tc.tile_pool(name="y_ps", bufs=1, space="PSUM"))

    # ---- constants / inputs to SBUF ----
    # Fused multi-chunk tiles: one DMA instruction covers many [128, .] chunks
    # (HWDGE descriptor-gen costs 625ns per DMA instruction, so instruction
    # count -- not bytes -- dominates the startup critical path).
    # wv and x share one DRAM buffer (host-concatenated) so the very first
    # vproj matmul's BOTH operands arrive in a single DMA chain
    wvxt = consts.tile([128, CC, 512 + T], BF16, tag="wvx")
    wqkt = consts.tile([128, CC, 1024], BF16, tag="wqk")
    wpt = consts.tile([128, NPAIR, 1024], BF16, tag="wp")
    wv = [wvxt[:, c, 0:512] for c in range(CC)]
    xT = [wvxt[:, c, 512:512 + T] for c in range(CC)]
    wqk = [wqkt[:, c, :] for c in range(CC)]
    wp = [wpt[:, j, :] for j in range(NPAIR)]
    bqk = consts.tile([128, 8], F32, tag="bqk")
    bv_row = consts.tile([1, 512], F32, tag="bv_row")
    bv_bc = consts.tile([128, 512], F32, tag="bv_bc")
    trieye = consts.tile([128, 2, 128], BF16, tag="trieye")
    tri = trieye[:, 0, :]
    eye = trieye[:, 1, :]

    # DMA order follows consumption: wv + x slab 0 (unlocks vproj 0-3) in
    # growing pieces, tiny consts, wqk (qkproj slab 0), remaining x, wp.
    wvxd = t_in["wvx"].rearrange("(c p) n -> p c n", p=128)
    wqkd = t_in["wqkT"].rearrange("(c p) n -> p c n", p=128)
    wpd = t_in["wpT"].rearrange("(j p) n -> p j n", p=128)
    for lo, hi in ((0, 1), (1, 3), (3, 5), (5, 8)):
        nc.sync.dma_start(wvxt[:, lo:hi, 0:1024], wvxd[:, lo:hi, 0:1024])
    nc.sync.dma_start(bv_row[:], t_in["bv"][:])
    nc.sync.dma_start(bqk[:], t_in["bqk"][:])
    nc.sync.dma_start(trieye[:], t_in["trieye"][:].rearrange("p (i n) -> p i n", i=2))
    nc.sync.dma_start(wqkt[:, 0:4, :], wqkd[:, 0:4, :])
    nc.sync.dma_start(wqkt[:, 4:8, :], wqkd[:, 4:8, :])
    nc.sync.dma_start(wvxt[:, :, 1024:512 + T], wvxd[:, :, 1024:512 + T])
    nc.sync.dma_start(wpt[:], wpd[:])
    nc.gpsimd.partition_broadcast(bv_bc[:], bv_row[:])

    # p-state warmup: the PE runs 2x slow until 3us of continuous execution;
    # spin dummy matmuls from t=0 so the ramp burns on garbage, not on the
    # first projection groups
    warm = consts.tile([128, 128], BF16, tag="warm")
    nc.vector.memset(warm[:], 1.0)
    for w in range(13):
        wps = mm_ps.tile([128, 512], F32, tag="mm", name="ps_w")
        for _ in range(4):
            nc.tensor.matmul(wps[:, 0:128], warm[:], warm[:],
                             start=True, stop=True)

    qk = [qkp.tile([128, T], BF16, tag=f"qk{j}", name=f"qk{j}") for j in range(CC)]
    v = [vp.tile([128, HPC, D + 1], BF16, tag=f"v{i}", name=f"v{i}") for i in range(NTT)]
    for i in range(NTT):
        nc.vector.memset(v[i][:, :, D:D + 1], 1.0)
    yt = [ytp.tile([128, T], BF16, tag=f"yt{j}", name=f"yt{j}") for j in range(NPAIR)]
    y_nat = {}

    def vproj_group(i):
        ps = mm_ps.tile([128, 512], F32, tag="mm", name="ps_v")
        for cc in range(CC):
            nc.tensor.matmul(
                ps[:],
                xT[cc][:, i * 128:(i + 1) * 128],
                wv[cc][:],
                start=(cc == 0),
                stop=(cc == CC - 1),
            )
        nc.vector.tensor_add(
            v[i][:, :, 0:D],
            ps[:].rearrange("p (h d) -> p h d", h=HPC),
            bv_bc[:].rearrange("p (h d) -> p h d", h=HPC),
        )

    def qkproj_group(oc, tt):
        ps = mm_ps.tile([128, 512], F32, tag="mm", name="ps_qk")
        for cc in range(CC):
            nc.tensor.matmul(
                ps[:],
                wqk[cc][:, oc * 128:(oc + 1) * 128],
                xT[cc][:, tt * 512:(tt + 1) * 512],
                start=(cc == 0),
                stop=(cc == CC - 1),
            )
        nc.vector.tensor_scalar_add(
            qk[oc][:, tt * 512:(tt + 1) * 512], ps[:], bqk[:, oc:oc + 1]
        )

    def geom(qt, kc):
        m = kc - 4 * qt  # >= 0 on diagonal chunks
        qoff = 128 * m if m > 0 else 0
        return m, qoff, 512 - qoff

    def scores_chunk(qt, hp, kc):
        """Scores + exp + mask for one (head-pair, key-chunk); returns e."""
        q0 = qt * 512
        m, qoff, nw = geom(qt, kc)
        k0 = kc * 128
        # the last pair's diagonal masks ride gpsimd (SBUF-only, legal):
        # they would otherwise queue on DVE ahead of the tail's norm chains
        meng = (nc.gpsimd if (qt == NQT - 1 and hp == NPAIR - 1 and
                              kc - 4 * qt >= 0) else nc.vector)
        sps = s_ps.tile([128, 1024], F32, tag="sps", name="sps")
        nc.tensor.matmul(
            sps[:, 0:nw],
            qk[4 + hp][0:64, k0:k0 + 128],
            qk[hp][0:64, q0 + qoff:q0 + 512],
            start=True, stop=True,
        )
        nc.tensor.matmul(
            sps[:, 512:512 + nw],
            qk[4 + hp][64:128, k0:k0 + 128],
            qk[hp][64:128, q0 + qoff:q0 + 512],
            start=True, stop=True,
        )
        e = ep.tile([128, 1024], BF16, tag="e", name="e")
        nc.scalar.activation(
            e[:].rearrange("p (i n) -> p i n", i=2)[:, :, 0:nw],
            sps[:].rearrange("p (i n) -> p i n", i=2)[:, :, 0:nw],
            mybir.ActivationFunctionType.Exp,
            scale=float(SCALE),
        )
        if m >= 0:
            meng.tensor_mul(
                e[:].rearrange("p (i n) -> p i n", i=2)[:, :, 0:128],
                e[:].rearrange("p (i n) -> p i n", i=2)[:, :, 0:128],
                tri[:].unsqueeze(1).broadcast_to([128, 2, 128]),
            )
        return e

    def ev_chunk(qt, hp, kc, ya, yb, e):
        """E@v with e as the stationary operand: per (head, 128-query slice)
        accumulate [128 q, 65] over key chunks; N=65 per matmul.

        PSUM start=True lazily zeroes the whole 2KB bank, so exactly ONE
        start per tile (first write poisons the bank; the other regions'
        first writes land on pending-zero bytes) and ONE stop on the bank's
        final write."""
        m, qoff, _ = geom(qt, kc)
        for h2, ydst in ((0, ya), (1, yb)):
            for qs in range(max(m, 0), 4):
                c0 = h2 * 512 + qs * 128 - qoff
                nc.tensor.matmul(
                    ydst[:, qs, :],
                    e[:, c0:c0 + 128],
                    v[kc][:, 2 * hp + h2, :],
                    start=(kc == 0 and qs == 0),
                    stop=(kc == 4 * qt + 3 and qs == 3),
                )

    def norm_pair(qt, hp, ya, yb):
        """Batched normalize of a whole pair: reciprocal of the denominator
        columns + broadcast multiply, PSUM -> y_nat bf16."""
        for h2, ysrc in ((0, ya), (1, yb)):
            h = 2 * hp + h2
            r = rp.tile([128, 4], F32, tag="r", name="r")
            nc.vector.reciprocal(r[:], ysrc[:, :, D])
            nc.vector.tensor_mul(
                y_nat[qt][:, :, h * D:(h + 1) * D],
                ysrc[:, :, 0:D],
                r[:].unsqueeze(2).broadcast_to([128, 4, D]),
            )

    def norm_qs(qt, hp, qs, ya, yb, on_act=False):
        """Per-query-slice normalize (tail of the last pair). The very last
        slice multiplies on ACT (idle once the exp chain drains) via an
        activation Copy with a per-partition scale."""
        for h2, ysrc in ((0, ya), (1, yb)):
            h = 2 * hp + h2
            r = rp.tile([128, 1], F32, tag="r1", name="r1")
            nc.vector.reciprocal(r[:], ysrc[:, qs, D:D + 1])
            if on_act:
                nc.scalar.activation(
                    y_nat[qt][:, qs, h * D:(h + 1) * D],
                    ysrc[:, qs, 0:D],
                    mybir.ActivationFunctionType.Copy,
                    scale=r[:],
                )
            else:
                nc.vector.tensor_mul(
                    y_nat[qt][:, qs, h * D:(h + 1) * D],
                    ysrc[:, qs, 0:D],
                    r[:].broadcast_to([128, D]),
                )

    def tdma(qt, qs, j):
        """yt[j] gets the transposed 128x128 block via the DMA XBAR."""
        nc.sync.dma_start(
            yt[j][:, qt * 512 + qs * 128:qt * 512 + (qs + 1) * 128],
            y_nat[qt][:, qs, j * 128:(j + 1) * 128],
            transpose=True,
        )

    def t_pe(qt, qs, j, on_act=False):
        """PE transpose + copy: lower latency than the DMA XBAR path."""
        ps = mm_ps.tile([128, 1024], BF16, tag="mm", name="ps_t")
        nc.tensor.transpose(ps[:, 0:128], y_nat[qt][:, qs, j * 128:(j + 1) * 128],
                            eye[:])
        dst = yt[j][:, qt * 512 + qs * 128:qt * 512 + (qs + 1) * 128]
        if on_act:
            nc.scalar.activation(dst, ps[:, 0:128],
                                 mybir.ActivationFunctionType.Copy)
        else:
            nc.vector.tensor_copy(dst, ps[:, 0:128])

    obuf = {}
    partials = {}

    def outproj_prerun(i, oh):
        """Accumulate pairs 0..2 of the final slab's output projection while
        the last pair's exp chain still runs; only the j=3 matmul and a DVE
        add remain after the last exp."""
        ps = mm_ps.tile([128, 512], F32, tag="mm", name="ps_pr")
        for j in range(NPAIR - 1):
            nc.tensor.matmul(
                ps[:],
                yt[j][:, i * 128:(i + 1) * 128],
                wp[j][:, oh * 512:(oh + 1) * 512],
                start=(j == 0),
                stop=(j == NPAIR - 2),
            )
        if i not in partials:
            partials[i] = outp.tile([128, 1024], BF16, tag="pp", name=f"pp{i}",
                                    bufs=4)
        nc.vector.tensor_copy(partials[i][:, oh * 512:(oh + 1) * 512], ps[:])

    def outproj_tail(i):
        """j3 matmul + partial add + ship, for one 128-token tile. The final
        tile (emitted after the last scores chunk) borrows a free scores-psum
        slot so both halves land in one [128,1024] tile -- one add, one DMA;
        earlier tiles would steal the slot from the still-running exp
        pipeline, so they go per-half through the mm rotation."""
        if i not in obuf:
            obuf[i] = outp.tile([128, 1024], BF16, tag="ob", name=f"ob{i}")
        ob = obuf[i]
        for oh in range(2):
            ps = mm_ps.tile([128, 512], F32, tag="mm", name="ps_tl")
            nc.tensor.matmul(
                ps[:],
                yt[NPAIR - 1][:, i * 128:(i + 1) * 128],
                wp[NPAIR - 1][:, oh * 512:(oh + 1) * 512],
                start=True, stop=True,
            )
            dst = ob[:, oh * 512:(oh + 1) * 512]
            nc.vector.tensor_add(dst, ps[:],
                                 partials[i][:, oh * 512:(oh + 1) * 512])
            nc.sync.dma_start(
                t_out[i * 128:(i + 1) * 128, oh * 512:(oh + 1) * 512],
                dst,
            )

    def outproj_group(i, oh):
        ps = mm_ps.tile([128, 512], F32, tag="mm", name="ps_op")
        for j in range(NPAIR):
            nc.tensor.matmul(
                ps[:],
                yt[j][:, i * 128:(i + 1) * 128],
                wp[j][:, oh * 512:(oh + 1) * 512],
                start=(j == 0),
                stop=(j == NPAIR - 1),
            )
        if i not in obuf:
            obuf[i] = outp.tile([128, 1024], BF16, tag="ob", name=f"ob{i}")
        ob = obuf[i]
        nc.vector.tensor_copy(ob[:, oh * 512:(oh + 1) * 512], ps[:])
        if i >= 4 * (NQT - 1):
            # final slab: ship each half as soon as its copy lands
            nc.sync.dma_start(
                t_out[i * 128:(i + 1) * 128, oh * 512:(oh + 1) * 512],
                ob[:, oh * 512:(oh + 1) * 512],
            )
        elif oh == 1:
            nc.sync.dma_start(t_out[i * 128:(i + 1) * 128, :], ob[:])

    # ---- schedule ----
    # Fill units per slab, sized to the slab's exp-vs-PE deficit. Each item
    # is (due, fn): the unit MUST be emitted before chunk-step `due` of its
    # slab (PE executes in program order, so a consumer emitted before its
    # producer deadlocks); due=None means "any time, flush by slab end".
    # Dues: vproj(g) feeds EV at pair-0 step kc=g; qkproj(oc=hp, tt) feeds
    # the one-ahead scores lookahead at step hp*L-1 of slab tt; k-halves
    # (oc=4+hp) feed the lookahead of pair hp's first slab-tt key chunk.
    def L(qt):
        return 4 * qt + 4

    fills = {
        0: [(15 if oc == 0 else None, lambda oc=oc: qkproj_group(oc, 1))
            for oc in range(CC)],
        1: ([(g, lambda i=i: vproj_group(i)) for g, i in
             zip(range(4, 8), range(4, 8))]
            + [(31 if oc == 0 else None, lambda oc=oc: qkproj_group(oc, 2))
               for oc in range(CC)]),
        2: ([(g, lambda i=i: vproj_group(i)) for g, i in
             zip(range(8, 12), range(8, 12))]
            + [(None, lambda i=i, oh=oh: outproj_group(i, oh))
               for i in range(0, 8) for oh in range(2)]
            + [(47, lambda: qkproj_group(0, 3))]),
        3: ([(11, lambda: qkproj_group(4, 3))]
            + [(g, lambda i=i: vproj_group(i)) for g, i in
               zip(range(12, 16), range(12, 16))]
            + [(hp * 16 - 1, lambda oc=hp: qkproj_group(oc, 3))
               for hp in (1, 2, 3)]
            + [(hp * 16 + 11, lambda oc=hp: qkproj_group(4 + oc, 3))
               for hp in (1, 2, 3)]),
    }
    # Late fill for slab 3's back half, where the fill deficit (ACT-paced
    # chunks vs small diagonal matmuls) is otherwise uncovered: outproj(2)
    # (ready since slab-2 end) through pairs 2-3, and the outproj(3)
    # pre-runs once pair 2's transposes have landed (after step 47).
    late_fills = {qt: [] for qt in range(NQT)}
    late_fills[3] = sorted(
        [(34 + 2 * k, fn) for k, fn in enumerate(
            lambda i=i, oh=oh: outproj_group(i, oh)
            for i in range(8, 12) for oh in range(2))],
        key=lambda p: p[0],
    )

    # Global chunk sequence; scores are pipelined one chunk ahead across pair
    # AND slab boundaries so the PE always has the next scores queued while
    # ACT digests the exp backlog.
    chunks = [(qt, hp, kc)
              for qt in range(NQT)
              for hp in range(NPAIR)
              for kc in range(4 * qt + 4)]
    nxt = {chunks[n]: chunks[n + 1] for n in range(len(chunks) - 1)}

    state = {}

    def chunk_step(qt, hp, kc):
        if not state:
            state["e"] = scores_chunk(qt, hp, kc)
        e = state["e"]
        if (qt, hp, kc) in nxt:
            state["e"] = scores_chunk(*nxt[(qt, hp, kc)])
        if kc == 0:
            state["y"] = (
                y_ps.tile([128, 4, D + 1], F32, tag="ya", name="ya", bufs=1),
                y_ps.tile([128, 4, D + 1], F32, tag="yb", name="yb", bufs=1),
            )
        ev_chunk(qt, hp, kc, state["y"][0], state["y"][1], e)

    def norm_qs(qt, hp, qs, ya, yb, on_act=False):
        """Per-query-slice normalize (tail of the last pair). The final
        slice multiplies on ACT (idle once the exp chain drains) via an
        activation Copy with a per-partition scale."""
        for h2, ysrc in ((0, ya), (1, yb)):
            h = 2 * hp + h2
            r = rp.tile([128, 1], F32, tag="r1", name="r1", bufs=8)
            nc.vector.reciprocal(r[:], ysrc[:, qs, D:D + 1])
            if on_act:
                nc.scalar.activation(
                    y_nat[qt][:, qs, h * D:(h + 1) * D],
                    ysrc[:, qs, 0:D],
                    mybir.ActivationFunctionType.Copy,
                    scale=r[:],
                )
            else:
                nc.vector.tensor_mul(
                    y_nat[qt][:, qs, h * D:(h + 1) * D],
                    ysrc[:, qs, 0:D],
                    r[:].broadcast_to([128, D]),
                )

    def post_chunk(qt, hp, kc):
        """Normalize/transpose work to emit right after a chunk. The last
        pair of the last slab staggers its per-slice tail: normalize at its
        diagonal chunk, transpose one chunk later, j3+add two chunks later,
        so each cross-engine chain's latency hides behind the next chunks
        instead of head-of-line blocking the in-order queues."""
        last_pair = (qt == NQT - 1 and hp == NPAIR - 1)
        if last_pair:
            qs = kc - 4 * qt
            if qs == -1:
                for i in range(12, 16):
                    for oh in range(2):
                        outproj_prerun(i, oh)
            if qs < 0:
                return
            if qs == 2:
                # tile 14's DVE adds are deferred past the final slice's
                # reciprocal so they don't stall the last ACT chain
                norm_qs(qt, hp, qs, *state["y"], on_act=True)
                t_pe(qt, qs, hp, on_act=True)
            elif qs == 3:
                norm_qs(qt, hp, qs, *state["y"], on_act=True)
                outproj_tail(4 * qt + 2)
                t_pe(qt, qs, hp, on_act=True)
                outproj_tail(4 * qt + 3)
            else:
                norm_qs(qt, hp, qs, *state["y"])
                t_pe(qt, qs, hp)
                outproj_tail(4 * qt + qs)
        elif kc == 4 * qt + 3:
            if qt not in y_nat:
                y_nat[qt] = ynp.tile([128, 4, 512], BF16, tag="yn",
                                     name=f"yn{qt}")
            norm_pair(qt, hp, *state["y"])
            for qs in range(4):
                tdma(qt, qs, hp)

    # prologue: vproj for slab-0 tokens, then qkproj slab 0
    for i in range(4):
        vproj_group(i)
    for oc in range(CC):
        qkproj_group(oc, 0)

    # main loop: per slab, interleave attention chunks with the fill list.
    # Forced emission for due items; otherwise paced evenly across the slab.
    for qt in range(NQT):
        steps = [(hp, kc) for hp in range(NPAIR) for kc in range(4 * qt + 4)]
        nstep = len(steps)
        fill = list(fills[qt])
        late = list(late_fills[qt])
        span_e = max(1, int(nstep * (0.62 if late else 1.0)))
        total = len(fill)
        emitted = 0
        for n, (hp, kc) in enumerate(steps):
            # forced: everything that must precede this chunk step
            keep = []
            for due, fn in fill:
                if due is not None and due <= n:
                    fn()
                    emitted += 1
                else:
                    keep.append((due, fn))
            fill = keep
            chunk_step(qt, hp, kc)
            post_chunk(qt, hp, kc)
            # paced: early list across the first span_e steps
            target = min(total, (total * (n + 1) + span_e - 1) // span_e)
            while emitted < target and fill:
                due, fn = fill.pop(0)
                fn()
                emitted += 1
            # late list: each item fires at its start step
            while late and late[0][0] <= n:
                late.pop(0)[1]()
        for due, fn in fill + late:
            fn()

    if t_in.get("dbg"):
        for j in range(NPAIR):
            nc.sync.dma_start(t_in["dbg_yt"][j * 128:(j + 1) * 128, :], yt[j][:])
        for qt in range(NQT):
            nc.sync.dma_start(
                t_in["dbg_yn"][qt * 128:(qt + 1) * 128, :],
                y_nat[qt][:].rearrange("p a b -> p (a b)"),
            )


def build_model():
    nc = bacc.Bacc(
        "TRN2",
        target_bir_lowering=False,
        debug=False,
        enable_asserts=False,
        num_devices=NCORES,
    )
    t_in = {
        "wvx": nc.dram_tensor("wvx", [C, 512 + T], BF16, kind="ExternalInput").ap(),
        "wqkT": nc.dram_tensor("wqkT", [C, 1024], BF16, kind="ExternalInput").ap(),
        "wpT": nc.dram_tensor("wpT", [512, C], BF16, kind="ExternalInput").ap(),
        "bqk": nc.dram_tensor("bqk", [128, 8], F32, kind="ExternalInput").ap(),
        "bv": nc.dram_tensor("bv", [1, 512], F32, kind="ExternalInput").ap(),
        "trieye": nc.dram_tensor("trieye", [128, 256], BF16, kind="ExternalInput").ap(),
    }
    t_out = nc.dram_tensor("out", [T, C], BF16, kind="ExternalOutput").ap()
    with tile.TileContext(nc) as tc:
        _attention_body(tc, t_in, t_out)
    nc.compile()
    return nc


def make_in_maps(x, w_attn, b_attn, w_proj):
    """Host-side sharding: per-core input dict for core (b, hg)."""
    trieye = np.concatenate(
        [np.triu(np.ones((128, 128), np.float32)), np.eye(128, dtype=np.float32)],
        axis=1,
    ).astype(NPBF16)
    in_maps = []
    xT_cache = {}
    for cid in range(NCORES):
        b, hg = cid // 2, cid % 2
        h0 = hg * HPC
        if b not in xT_cache:
            xT_cache[b] = np.ascontiguousarray(x[b].T).astype(NPBF16)
        rq = slice(h0 * D, (h0 + HPC) * D)
        rk = slice(C + h0 * D, C + (h0 + HPC) * D)
        rv = slice(2 * C + h0 * D, 2 * C + (h0 + HPC) * D)
        wqkT = np.ascontiguousarray(
            np.concatenate([w_attn[rq], w_attn[rk]], axis=0).T
        ).astype(NPBF16)
        wvT = np.ascontiguousarray(w_attn[rv].T).astype(NPBF16)
        wpT = np.ascontiguousarray(w_proj[:, h0 * D:(h0 + HPC) * D].T).astype(NPBF16)
        bqk = np.stack(
            [b_attn[rq].reshape(4, 128)[j] for j in range(4)]
            + [b_attn[rk].reshape(4, 128)[j] for j in range(4)],
            axis=1,
        ).astype(np.float32)
        bv = b_attn[rv].reshape(1, 512).astype(np.float32)
        in_maps.append({
            "wvx": np.ascontiguousarray(
                np.concatenate([wvT, xT_cache[b]], axis=1)),
            "wqkT": wqkT,
            "wpT": wpT,
            "bqk": np.ascontiguousarray(bqk),
            "bv": bv,
            "trieye": trieye,
        })
    return in_maps


_NC_CACHE = []


def kernel(x, w_attn, b_attn, w_proj, b_proj):
    x = np.asarray(x, dtype=np.float32)
    w_attn = np.asarray(w_attn, dtype=np.float32)
    b_attn = np.asarray(b_attn, dtype=np.float32)
    w_proj = np.asarray(w_proj, dtype=np.float32)
    b_proj = np.asarray(b_proj, dtype=np.float32)

    if not _NC_CACHE:
        _NC_CACHE.append(build_model())
    nc = _NC_CACHE[0]
    in_maps = make_in_maps(x, w_attn, b_attn, w_proj)
    res = None
    for attempt in range(3):
        try:
            res = run_bass_kernel_spmd(nc, in_maps, core_ids=list(range(NCORES)))
            break
        except Exception:
            if attempt == 2:
                raise
            import time
            time.sleep(5)
    out = np.empty((B, T, C), np.float32)
    for b in range(B):
        out[b] = (res.results[2 * b]["out"].astype(np.float32)
                  + res.results[2 * b + 1]["out"].astype(np.float32))
    out += b_proj[None, None, :]
    return out
